# revision 16
# baseline (speedup 1.0000x reference)
"""Tensor-parallel Llama GQA attention layer (B=1, S=2048, D=2048, H=32, KV=8)
for 8 Trainium2 NeuronCores.

Sharding: one KV group per core (kv head g + its 4 q heads). Each core computes
its heads' attention and a partial out-projection (contraction over its 256
head-dim columns of wo); the host sums the 8 fp16 partials (the TP all-reduce)
and transposes back to [1, S, D].

On-core layout is feature-major (transposed): xt=[D,S], QT=[j,S], KT/VT=[hd,S].
Scores are built per (head-pair, s-superblock of 512, t-block of 128) as
ST=[t,s] tiles; softmax is unnormalized exp (scores are O(1) so no max
subtraction is needed) with the denominator obtained by ones columns in the
V stationary tile, and a single normalization divide at the end.

Schedule notes: emission order is the Tile scheduler's priority, so the
program interleaves projection chunks, attention superblocks and partial
out-projections (proj0, att0, proj1, att1, oproj0, proj2, att2, oproj1,
proj3, att3, oproj2, oproj3) to keep PE (the bottleneck engine) saturated
and at full p-state. Diagonal 128x512 score blocks are column-narrowed to
the causal region for MM1/exp/MM2, and the 0/1 mask multiply only touches
the 128-col triangular band (one shared tril tile). The softmax denominator
rows land inside the ut PSUM tile (ones col after V for the even head, before
V for the odd head, whose output sits at partitions 63:128), so normalization
is 2 reciprocals + a K=1 PE broadcast matmul + 2 partition-aligned DVE
multiplies - no partition-shift DMAs.
"""

import numpy as np
import ml_dtypes

S = 2048
D = 2048
H = 32
KV = 8
HD = 64
R = 4  # heads per kv group
NC = 8  # cores

BF16 = ml_dtypes.bfloat16
FP16 = np.float16


def _build_program(debug_dump=False):
    import concourse.mybir as mybir
    import concourse.tile as tile
    from concourse import bacc

    f32 = mybir.dt.float32
    f16 = mybir.dt.float16
    bf16 = mybir.dt.bfloat16

    nc = bacc.Bacc("TRN2", debug=False, num_devices=NC)
    dbg = {}
    if debug_dump:
        dbg["qtr"] = nc.dram_tensor("dbg_qtr", [128, 2, S], bf16, kind="ExternalOutput")
        dbg["kv2"] = nc.dram_tensor("dbg_kv2", [128, S], bf16, kind="ExternalOutput")
        dbg["vext"] = nc.dram_tensor("dbg_vext", [128, S // 128, 65], bf16, kind="ExternalOutput")
        dbg["at"] = nc.dram_tensor("dbg_at", [128, 2, S], bf16, kind="ExternalOutput")
        dbg["utsb0"] = nc.dram_tensor("dbg_utsb0", [65, 2, 512], f32, kind="ExternalOutput")
        dbg["rc0"] = nc.dram_tensor("dbg_rc0", [64, 2, 512], f32, kind="ExternalOutput")
        dbg["et0"] = nc.dram_tensor("dbg_et0", [128, 2, 512], bf16, kind="ExternalOutput")

    xt = nc.dram_tensor("xt", [D, S], bf16, kind="ExternalInput")
    wq_t = nc.dram_tensor("wq_t", [D, R * HD], bf16, kind="ExternalInput")
    wkv_t = nc.dram_tensor("wkv_t", [D, 2 * HD], bf16, kind="ExternalInput")
    wo_t = nc.dram_tensor("wo_t", [R * HD, D], bf16, kind="ExternalInput")
    cosb = nc.dram_tensor("cosb", [128, S], bf16, kind="ExternalInput")
    sinb = nc.dram_tensor("sinb", [128, S], bf16, kind="ExternalInput")
    tril = nc.dram_tensor("tril", [128, 128], bf16, kind="ExternalInput")
    ident64 = nc.dram_tensor("ident64", [64, 64], bf16, kind="ExternalInput")
    out_t = nc.dram_tensor("out_t", [D, S], f16, kind="ExternalOutput")

    DT = D // 128  # 16 d tiles
    TT = S // 128  # 16 t blocks
    SB = S // 512  # 4 s superblocks

    with tile.TileContext(nc) as tc:
        with (
            tc.tile_pool(name="persist", bufs=1) as persist,
            tc.tile_pool(name="qstage", bufs=2) as qstage_p,
            tc.tile_pool(name="rtmp", bufs=2) as rtmp_p,
            tc.tile_pool(name="et", bufs=6) as etp,
            tc.tile_pool(name="rcp", bufs=2) as rcp_p,
            tc.tile_pool(name="ostage", bufs=3) as ostage_p,
            tc.tile_pool(name="st_ps", bufs=2, space="PSUM") as st_ps,
            tc.tile_pool(name="ut_ps", bufs=1, space="PSUM") as ut_ps,
            tc.tile_pool(name="qkv_ps", bufs=2, space="PSUM") as qkv_ps,
        ):
            # ---- persistent SBUF tensors ----
            xt_sb = persist.tile([128, DT, S], bf16)
            wq_sb = persist.tile([128, DT, R * HD], bf16)
            wkv_sb = persist.tile([128, DT, 2 * HD], bf16)
            wo_sb = persist.tile([128, 2, D], bf16)
            cos_sb = persist.tile([128, S], bf16)
            sin_sb = persist.tile([128, S], bf16)
            tril_sb = persist.tile([128, 128], bf16)
            ident_sb = persist.tile([128, 64], bf16)
            ones_sb = persist.tile([128, 64], bf16)

            qtr_sb = persist.tile([128, 2, S], bf16)   # roped Q, head-major
            kv2_sb = persist.tile([128, S], bf16)      # 0:64 roped K, 64:128 VT
            ko_sb = persist.tile([128, S], bf16)       # 64:128 roped K (odd heads)
            # V blocks [t, hd]: cols 0:64 = V, col 64 = ones (denominator)
            vext_sb = persist.tile([128, TT, 65], bf16)
            at_sb = persist.tile([128, 2, S], bf16)    # normalized attn out

            # ---- input DMA, ordered for the pipeline lead-in ----
            wkv_r = wkv_t.ap().rearrange("(dt p) j -> p dt j", p=128)
            wq_r = wq_t.ap().rearrange("(dt p) j -> p dt j", p=128)
            xt_r = xt.ap().rearrange("(dt p) s -> p dt s", p=128)
            out_r = out_t.ap().rearrange("(g p) s -> p g s", p=128)

            nc.sync.dma_start(out=wkv_sb, in_=wkv_r)
            nc.sync.dma_start(out=ident_sb[64:128, :], in_=ident64.ap())
            nc.sync.dma_start(out=xt_sb[:, 0:8, 0:512], in_=xt_r[:, 0:8, 0:512])
            nc.sync.dma_start(out=xt_sb[:, 8:16, 0:512], in_=xt_r[:, 8:16, 0:512])
            nc.sync.dma_start(out=cos_sb[:, 0:512], in_=cosb.ap()[:, 0:512])
            nc.sync.dma_start(out=sin_sb[:, 0:512], in_=sinb.ap()[:, 0:512])
            nc.sync.dma_start(out=wq_sb, in_=wq_r)
            nc.sync.dma_start(out=tril_sb, in_=tril.ap())
            for sc in range(1, 4):
                c0, c1 = sc * 512, (sc + 1) * 512
                nc.sync.dma_start(out=xt_sb[:, :, c0:c1], in_=xt_r[:, :, c0:c1])
                nc.sync.dma_start(out=cos_sb[:, c0:c1], in_=cosb.ap()[:, c0:c1])
                nc.sync.dma_start(out=sin_sb[:, c0:c1], in_=sinb.ap()[:, c0:c1])
            for jt in range(2):
                nc.sync.dma_start(out=wo_sb[:, jt, :], in_=wo_t.ap()[jt * 128:(jt + 1) * 128, :])

            nc.vector.memset(vext_sb, 1.0)  # ones cols 0 and 65 stay
            nc.vector.memset(ones_sb, 1.0)

            # ---- RoPE on a 512-col chunk ----
            # within each 64-row head block: rows 0:32 even comps, 32:64 odd
            # comps; roped = q*C + swap(q)*S (C=[cos x4], S=[-sin,+sin]x2)
            def rope_chunk(src, dst, nrows, c0, c1):
                swp = rtmp_p.tile([128, 512], bf16, tag="swap")
                for b in range(nrows // 64):
                    nc.gpsimd.dma_start(out=swp[b * 64:b * 64 + 32, :], in_=src[b * 64 + 32:b * 64 + 64, c0:c1])
                    nc.gpsimd.dma_start(out=swp[b * 64 + 32:b * 64 + 64, :], in_=src[b * 64:b * 64 + 32, c0:c1])
                t1 = rtmp_p.tile([128, 512], bf16, tag="ropetmp")
                nc.vector.tensor_mul(t1[:nrows], src[:nrows, c0:c1], cos_sb[0:nrows, c0:c1])
                nc.vector.tensor_mul(swp[:nrows], swp[:nrows], sin_sb[0:nrows, c0:c1])
                nc.vector.tensor_add(dst, t1[:nrows], swp[:nrows])

            kstage = qstage_p.tile([64, S], bf16, tag="kstage")
            qstage_tiles = [
                qstage_p.tile([128, S], bf16, tag="qstage", name=f"qst{i}")
                for i in range(2)
            ]

            # ---- projection chunk: KV proj + K rope + V transpose + Q proj ----
            def proj_chunk(si):
                c0, c1 = si * 512, (si + 1) * 512
                ps = qkv_ps.tile([128, 512], f32, tag="mm")
                for dt in range(DT):
                    nc.tensor.matmul(
                        ps, wkv_sb[:, dt, :], xt_sb[:, dt, c0:c1],
                        start=(dt == 0), stop=(dt == DT - 1),
                    )
                nc.vector.tensor_copy(kstage[:, c0:c1], ps[0:64, :])
                nc.vector.tensor_copy(kv2_sb[64:128, c0:c1], ps[64:128, :])
                rope_chunk(kstage, kv2_sb[0:64, c0:c1], 64, c0, c1)
                # roped K copy at base partition 64 (odd heads), V transposes
                nc.gpsimd.dma_start(out=ko_sb[64:128, c0:c1], in_=kv2_sb[0:64, c0:c1])
                for tt in range(4 * si, 4 * si + 4):
                    vps = st_ps.tile([128, 64], bf16, tag="st")
                    nc.tensor.transpose(vps, kv2_sb[64:128, tt * 128:(tt + 1) * 128], ident_sb[64:128, :])
                    nc.vector.tensor_copy(vext_sb[:, tt, 0:64], vps)
                for jt in range(2):
                    qst = qstage_tiles[jt]
                    ps = qkv_ps.tile([128, 512], f32, tag="mm")
                    for dt in range(DT):
                        nc.tensor.matmul(
                            ps, wq_sb[:, dt, jt * 128:(jt + 1) * 128], xt_sb[:, dt, c0:c1],
                            start=(dt == 0), stop=(dt == DT - 1),
                        )
                    nc.vector.tensor_copy(qst[:, c0:c1], ps)
                    rope_chunk(qst, qtr_sb[:, jt, c0:c1], 128, c0, c1)

            # ---- attention superblock: heads in pairs (2jt, 2jt+1) ----
            # even head scores at PE contraction rows 0:64, odd at 64:128 ->
            # MM1s overlap in disjoint row strips.
            def att(si):
                nblk = 4 * (si + 1)
                c0, c1 = si * 512, (si + 1) * 512
                for jt in range(2):
                    ut = ut_ps.tile([65, 2, 512], f32, tag="ut")
                    for j in range(nblk):
                        jj = j - 4 * si  # >=0 on the diagonal superblock
                        cc0 = 128 * jj if jj > 0 else 0  # causal col narrowing
                        st2 = st_ps.tile([128, 2, 512], f32, tag="st")
                        nc.tensor.matmul(
                            st2[:, 0, cc0:512],
                            kv2_sb[0:64, j * 128:(j + 1) * 128],
                            qtr_sb[0:64, jt, c0 + cc0:c1],
                            start=True, stop=True,
                        )
                        nc.tensor.matmul(
                            st2[:, 1, cc0:512],
                            ko_sb[64:128, j * 128:(j + 1) * 128],
                            qtr_sb[64:128, jt, c0 + cc0:c1],
                            start=True, stop=True,
                        )
                        et2 = etp.tile([128, 2, 512], bf16, tag="et")
                        nc.scalar.activation(
                            et2[:, :, cc0:512], st2[:, :, cc0:512],
                            mybir.ActivationFunctionType.Exp)
                        if jj >= 0:
                            # triangular band: cols [cc0, cc0+128)
                            nc.vector.tensor_mul(
                                et2[:, 0, cc0:cc0 + 128], et2[:, 0, cc0:cc0 + 128], tril_sb)
                            nc.vector.tensor_mul(
                                et2[:, 1, cc0:cc0 + 128], et2[:, 1, cc0:cc0 + 128], tril_sb)
                        if debug_dump and si == 0 and jt == 0 and j == 0:
                            nc.sync.dma_start(out=dbg["et0"].ap(), in_=et2)
                        # per head: rows 0:64 = V out, row 64 = denominator
                        nc.tensor.matmul(
                            ut[:, 0, cc0:512], vext_sb[:, j, 0:65], et2[:, 0, cc0:512],
                            start=(j == 0), stop=(j == nblk - 1),
                        )
                        nc.tensor.matmul(
                            ut[:, 1, cc0:512], vext_sb[:, j, 0:65], et2[:, 1, cc0:512],
                            start=(j == 0), stop=(j == nblk - 1),
                        )
                    # normalize: at = v_out * (1/den). Evacuate ut whole
                    # (frees the PSUM slot), broadcast the raw den rows to 64
                    # partitions via a base-0 K=1 PE matmul, then reciprocal
                    # at partition base 0 and two partition-aligned muls.
                    utsb = rcp_p.tile([65, 2, 512], f32, tag="utsb")
                    nc.vector.tensor_copy(utsb, ut)
                    denb = rcp_p.tile([1, 2, 512], bf16, tag="denb")
                    nc.vector.tensor_copy(denb, utsb[64:65, :, :])
                    db0 = qkv_ps.tile([64, 512], f32, tag="mm")
                    db1 = qkv_ps.tile([64, 512], f32, tag="mm")
                    nc.tensor.matmul(db0, ones_sb[0:1, 0:64], denb[0:1, 0, :],
                                     start=True, stop=True)
                    nc.tensor.matmul(db1, ones_sb[0:1, 0:64], denb[0:1, 1, :],
                                     start=True, stop=True)
                    rc64 = rcp_p.tile([64, 2, 512], f32, tag="rc64")
                    nc.vector.reciprocal_approx_fast(rc64[:, 0, :], db0)
                    nc.vector.reciprocal_approx_fast(rc64[:, 1, :], db1)
                    nc.vector.tensor_mul(at_sb[0:64, jt, c0:c1], utsb[0:64, 0, :], rc64[:, 0, :])
                    tmp64 = rcp_p.tile([64, 512], bf16, tag="tmp64")
                    nc.vector.tensor_mul(tmp64, utsb[0:64, 1, :], rc64[:, 1, :])
                    nc.gpsimd.dma_start(out=at_sb[64:128, jt, c0:c1], in_=tmp64)
                    if debug_dump and si == 0 and jt == 0:
                        nc.sync.dma_start(out=dbg["utsb0"].ap(), in_=utsb)
                        nc.sync.dma_start(out=dbg["rc0"].ap(), in_=rc64)

            # ---- partial out-projection for superblock si ----
            def oproj(si):
                c0, c1 = si * 512, (si + 1) * 512
                ost = None
                for dt in range(DT):
                    po = qkv_ps.tile([128, 512], f32, tag="mm")
                    for jt in range(2):
                        nc.tensor.matmul(
                            po, wo_sb[:, jt, dt * 128:(dt + 1) * 128],
                            at_sb[:, jt, c0:c1],
                            start=(jt == 0), stop=(jt == 1),
                        )
                    if dt % 4 == 0:
                        ost = ostage_p.tile([128, 4, 512], f16, tag="ost")
                    if dt % 4 == 1:
                        nc.scalar.activation(ost[:, dt % 4, :], po, mybir.ActivationFunctionType.Copy)
                    else:
                        nc.vector.tensor_copy(ost[:, dt % 4, :], po)
                    if dt % 4 == 3:
                        g = dt - 3
                        nc.sync.dma_start(
                            out=out_r[:, g:g + 4, c0:c1], in_=ost)

            # ---- emission order == scheduler priority ----
            proj_chunk(0)
            att(0)
            proj_chunk(1)
            att(1)
            oproj(0)
            proj_chunk(2)
            att(2)
            oproj(1)
            proj_chunk(3)
            att(3)
            oproj(2)
            oproj(3)

            if debug_dump:
                nc.sync.dma_start(out=dbg["qtr"].ap(), in_=qtr_sb)
                nc.sync.dma_start(out=dbg["kv2"].ap(), in_=kv2_sb)
                nc.sync.dma_start(out=dbg["vext"].ap(), in_=vext_sb)
                nc.sync.dma_start(out=dbg["at"].ap(), in_=at_sb)

    nc.compile()
    return nc


_SIGMA = np.concatenate([np.arange(0, HD, 2), np.arange(1, HD, 2)])


def _prep_inputs(x, freqs_cis, wq, wk, wv, wo):
    """Host-side shard + layout prep. Returns per-core in_maps."""
    x = np.asarray(x, np.float32).reshape(S, D)
    freqs_cis = np.asarray(freqs_cis, np.float32)
    wq = np.asarray(wq, np.float32)
    wk = np.asarray(wk, np.float32)
    wv = np.asarray(wv, np.float32)
    wo = np.asarray(wo, np.float32)

    xt = np.ascontiguousarray(x.T).astype(BF16)

    cosT = np.ascontiguousarray(freqs_cis[:, :, 0].T)  # [32, S]
    sinT = np.ascontiguousarray(freqs_cis[:, :, 1].T)
    cosb = np.ascontiguousarray(np.tile(cosT, (4, 1))).astype(BF16)
    sinb = np.ascontiguousarray(
        np.concatenate([-sinT, sinT, -sinT, sinT], 0)).astype(BF16)

    tloc = np.arange(128)[:, None]
    cloc = np.arange(128)[None, :]
    tril = (tloc <= cloc).astype(np.float32).astype(BF16)
    ident64 = np.eye(64, dtype=np.float32).astype(BF16)

    scale = 1.0 / np.sqrt(HD)
    in_maps = []
    for g in range(NC):
        wqg = wq[g * R * HD:(g + 1) * R * HD].reshape(R, HD, D)[:, _SIGMA, :].reshape(R * HD, D)
        wq_tg = np.ascontiguousarray(wqg.T).astype(BF16)
        wkg = wk[g * HD:(g + 1) * HD][_SIGMA] * scale
        wvg = wv[g * HD:(g + 1) * HD]
        wkv_tg = np.ascontiguousarray(np.concatenate([wkg, wvg], 0).T).astype(BF16)
        wo_tg = np.ascontiguousarray(wo[:, g * R * HD:(g + 1) * R * HD].T).astype(BF16)
        in_maps.append({
            "xt": xt,
            "wq_t": wq_tg,
            "wkv_t": wkv_tg,
            "wo_t": wo_tg,
            "cosb": cosb,
            "sinb": sinb,
            "tril": tril,
            "ident64": ident64,
        })
    return in_maps


_CACHED = {}


def _get_program():
    if "nc" not in _CACHED:
        _CACHED["nc"] = _build_program()
    return _CACHED["nc"]


def kernel(x, freqs_cis, wq, wk, wv, wo, _trace=False):
    from concourse.bass_utils import run_bass_kernel_spmd

    nc = _get_program()
    in_maps = _prep_inputs(x, freqs_cis, wq, wk, wv, wo)
    res = run_bass_kernel_spmd(nc, in_maps, core_ids=list(range(NC)), trace=_trace)
    acc = np.zeros((D, S), np.float64)
    for c in range(NC):
        acc += res.results[c]["out_t"].astype(np.float64)
    out = np.ascontiguousarray(acc.T, dtype=np.float32).reshape(1, S, D)
    if _trace:
        return out, res
    return out


# revision 19
# speedup vs baseline: 1.0140x; 1.0140x over previous
"""Tensor-parallel Llama GQA attention layer (B=1, S=2048, D=2048, H=32, KV=8)
for 8 Trainium2 NeuronCores.

Sharding: one KV group per core (kv head g + its 4 q heads). Each core computes
its heads' attention and a partial out-projection (contraction over its 256
head-dim columns of wo); the host sums the 8 fp16 partials (the TP all-reduce)
and transposes back to [1, S, D].

On-core layout is feature-major (transposed): xt=[D,S], QT=[j,S], KT/VT=[hd,S].
Scores are built per (head-pair, s-superblock of 512, t-block of 128) as
ST=[t,s] tiles; softmax is unnormalized exp (scores are O(1) so no max
subtraction is needed) with the denominator obtained by ones columns in the
V stationary tile, and a single normalization divide at the end.

Schedule notes: emission order is the Tile scheduler's priority, so the
program interleaves projection chunks, attention superblocks and partial
out-projections (proj0, att0, proj1, att1, oproj0, proj2, att2, oproj1,
proj3, att3, oproj2, oproj3) to keep PE (the bottleneck engine) saturated
and at full p-state. Diagonal 128x512 score blocks are column-narrowed to
the causal region for MM1/exp/MM2, and the 0/1 mask multiply only touches
the 128-col triangular band (one shared tril tile). The softmax denominator
rows land inside the ut PSUM tile (ones col after V for the even head, before
V for the odd head, whose output sits at partitions 63:128), so normalization
is 2 reciprocals + a K=1 PE broadcast matmul + 2 partition-aligned DVE
multiplies - no partition-shift DMAs.
"""

import numpy as np
import ml_dtypes

S = 2048
D = 2048
H = 32
KV = 8
HD = 64
R = 4  # heads per kv group
NC = 8  # cores

BF16 = ml_dtypes.bfloat16
FP16 = np.float16


def _build_program(debug_dump=False):
    import concourse.mybir as mybir
    import concourse.tile as tile
    from concourse import bacc

    f32 = mybir.dt.float32
    f16 = mybir.dt.float16
    bf16 = mybir.dt.bfloat16

    nc = bacc.Bacc("TRN2", debug=False, num_devices=NC)
    dbg = {}
    if debug_dump:
        dbg["qtr"] = nc.dram_tensor("dbg_qtr", [128, 2, S], bf16, kind="ExternalOutput")
        dbg["kv2"] = nc.dram_tensor("dbg_kv2", [128, S], bf16, kind="ExternalOutput")
        dbg["vext"] = nc.dram_tensor("dbg_vext", [128, S // 128, 65], bf16, kind="ExternalOutput")
        dbg["at"] = nc.dram_tensor("dbg_at", [128, 2, S], bf16, kind="ExternalOutput")
        dbg["rc0"] = nc.dram_tensor("dbg_rc0", [64, 2, 512], f32, kind="ExternalOutput")
        dbg["et0"] = nc.dram_tensor("dbg_et0", [128, 2, 512], bf16, kind="ExternalOutput")

    xt = nc.dram_tensor("xt", [D, S], bf16, kind="ExternalInput")
    wq_t = nc.dram_tensor("wq_t", [D, R * HD], bf16, kind="ExternalInput")
    wkv_t = nc.dram_tensor("wkv_t", [D, 2 * HD], bf16, kind="ExternalInput")
    wo_t = nc.dram_tensor("wo_t", [R * HD, D], bf16, kind="ExternalInput")
    cosb = nc.dram_tensor("cosb", [128, S], bf16, kind="ExternalInput")
    sinb = nc.dram_tensor("sinb", [128, S], bf16, kind="ExternalInput")
    tril = nc.dram_tensor("tril", [128, 128], bf16, kind="ExternalInput")
    ident64 = nc.dram_tensor("ident64", [64, 64], bf16, kind="ExternalInput")
    out_t = nc.dram_tensor("out_t", [D, S], f16, kind="ExternalOutput")

    DT = D // 128  # 16 d tiles
    TT = S // 128  # 16 t blocks
    SB = S // 512  # 4 s superblocks

    with tile.TileContext(nc) as tc:
        with (
            tc.tile_pool(name="persist", bufs=1) as persist,
            tc.tile_pool(name="qstage", bufs=2) as qstage_p,
            tc.tile_pool(name="rtmp", bufs=2) as rtmp_p,
            tc.tile_pool(name="et", bufs=6) as etp,
            tc.tile_pool(name="rcp", bufs=2) as rcp_p,
            tc.tile_pool(name="ostage", bufs=3) as ostage_p,
            tc.tile_pool(name="st_ps", bufs=2, space="PSUM") as st_ps,
            tc.tile_pool(name="ut_ps", bufs=1, space="PSUM") as ut_ps,
            tc.tile_pool(name="qkv_ps", bufs=2, space="PSUM") as qkv_ps,
        ):
            # ---- persistent SBUF tensors ----
            xt_sb = persist.tile([128, DT, S], bf16)
            wq_sb = persist.tile([128, DT, R * HD], bf16)
            wkv_sb = persist.tile([128, DT, 2 * HD], bf16)
            wo_sb = persist.tile([128, 2, D], bf16)
            cos_sb = persist.tile([128, S], bf16)
            sin_sb = persist.tile([128, S], bf16)
            tril_sb = persist.tile([128, 128], bf16)
            ident_sb = persist.tile([128, 64], bf16)
            ones_sb = persist.tile([128, 64], bf16)

            qtr_sb = persist.tile([128, 2, S], bf16)   # roped Q, head-major
            kv2_sb = persist.tile([128, S], bf16)      # 0:64 roped K, 64:128 VT
            ko_sb = persist.tile([128, S], bf16)       # 64:128 roped K (odd heads)
            # V blocks [t, hd]: cols 0:64 = V, col 64 = ones (denominator)
            vext_sb = persist.tile([128, TT, 65], bf16)
            at_sb = persist.tile([128, 2, S], bf16)    # normalized attn out

            # ---- input DMA, ordered for the pipeline lead-in ----
            wkv_r = wkv_t.ap().rearrange("(dt p) j -> p dt j", p=128)
            wq_r = wq_t.ap().rearrange("(dt p) j -> p dt j", p=128)
            xt_r = xt.ap().rearrange("(dt p) s -> p dt s", p=128)
            out_r = out_t.ap().rearrange("(g p) s -> p g s", p=128)

            nc.sync.dma_start(out=wkv_sb, in_=wkv_r)
            nc.sync.dma_start(out=ident_sb[64:128, :], in_=ident64.ap())
            nc.sync.dma_start(out=xt_sb[:, 0:8, 0:512], in_=xt_r[:, 0:8, 0:512])
            nc.sync.dma_start(out=xt_sb[:, 8:16, 0:512], in_=xt_r[:, 8:16, 0:512])
            nc.sync.dma_start(out=cos_sb[:, 0:512], in_=cosb.ap()[:, 0:512])
            nc.sync.dma_start(out=sin_sb[:, 0:512], in_=sinb.ap()[:, 0:512])
            nc.sync.dma_start(out=wq_sb, in_=wq_r)
            nc.sync.dma_start(out=tril_sb, in_=tril.ap())
            for sc in range(1, 4):
                c0, c1 = sc * 512, (sc + 1) * 512
                nc.sync.dma_start(out=xt_sb[:, :, c0:c1], in_=xt_r[:, :, c0:c1])
                nc.sync.dma_start(out=cos_sb[:, c0:c1], in_=cosb.ap()[:, c0:c1])
                nc.sync.dma_start(out=sin_sb[:, c0:c1], in_=sinb.ap()[:, c0:c1])
            for jt in range(2):
                nc.sync.dma_start(out=wo_sb[:, jt, :], in_=wo_t.ap()[jt * 128:(jt + 1) * 128, :])

            nc.vector.memset(vext_sb, 1.0)  # ones cols 0 and 65 stay
            nc.vector.memset(ones_sb, 1.0)

            # ---- RoPE on a 512-col chunk ----
            # within each 64-row head block: rows 0:32 even comps, 32:64 odd
            # comps; roped = q*C + swap(q)*S (C=[cos x4], S=[-sin,+sin]x2)
            def rope_chunk(src, dst, nrows, c0, c1):
                swp = rtmp_p.tile([128, 512], bf16, tag="swap")
                for b in range(nrows // 64):
                    nc.gpsimd.dma_start(out=swp[b * 64:b * 64 + 32, :], in_=src[b * 64 + 32:b * 64 + 64, c0:c1])
                    nc.gpsimd.dma_start(out=swp[b * 64 + 32:b * 64 + 64, :], in_=src[b * 64:b * 64 + 32, c0:c1])
                t1 = rtmp_p.tile([128, 512], bf16, tag="ropetmp")
                nc.vector.tensor_mul(t1[:nrows], src[:nrows, c0:c1], cos_sb[0:nrows, c0:c1])
                nc.vector.tensor_mul(swp[:nrows], swp[:nrows], sin_sb[0:nrows, c0:c1])
                nc.vector.tensor_add(dst, t1[:nrows], swp[:nrows])

            kstage = qstage_p.tile([64, S], bf16, tag="kstage")
            qstage_tiles = [
                qstage_p.tile([128, S], bf16, tag="qstage", name=f"qst{i}")
                for i in range(2)
            ]

            # ---- projection chunk: KV proj + K rope + V transpose + Q proj ----
            def proj_chunk(si):
                c0, c1 = si * 512, (si + 1) * 512
                ps = qkv_ps.tile([128, 512], f32, tag="mm")
                for dt in range(DT):
                    nc.tensor.matmul(
                        ps, wkv_sb[:, dt, :], xt_sb[:, dt, c0:c1],
                        start=(dt == 0), stop=(dt == DT - 1),
                    )
                nc.vector.tensor_copy(kstage[:, c0:c1], ps[0:64, :])
                nc.vector.tensor_copy(kv2_sb[64:128, c0:c1], ps[64:128, :])
                rope_chunk(kstage, kv2_sb[0:64, c0:c1], 64, c0, c1)
                # roped K copy at base partition 64 (odd heads), V transposes
                nc.gpsimd.dma_start(out=ko_sb[64:128, c0:c1], in_=kv2_sb[0:64, c0:c1])
                for tt in range(4 * si, 4 * si + 4):
                    vps = st_ps.tile([128, 64], bf16, tag="st")
                    nc.tensor.transpose(vps, kv2_sb[64:128, tt * 128:(tt + 1) * 128], ident_sb[64:128, :])
                    nc.vector.tensor_copy(vext_sb[:, tt, 0:64], vps)
                for jt in range(2):
                    qst = qstage_tiles[jt]
                    ps = qkv_ps.tile([128, 512], f32, tag="mm")
                    for dt in range(DT):
                        nc.tensor.matmul(
                            ps, wq_sb[:, dt, jt * 128:(jt + 1) * 128], xt_sb[:, dt, c0:c1],
                            start=(dt == 0), stop=(dt == DT - 1),
                        )
                    nc.vector.tensor_copy(qst[:, c0:c1], ps)
                    rope_chunk(qst, qtr_sb[:, jt, c0:c1], 128, c0, c1)

            # ---- attention superblock: heads in pairs (2jt, 2jt+1) ----
            # even head scores at PE contraction rows 0:64, odd at 64:128 ->
            # MM1s overlap in disjoint row strips.
            def att(si):
                nblk = 4 * (si + 1)
                c0, c1 = si * 512, (si + 1) * 512
                for jt in range(2):
                    ut = ut_ps.tile([65, 2, 512], f32, tag="ut")
                    for j in range(nblk):
                        jj = j - 4 * si  # >=0 on the diagonal superblock
                        cc0 = 128 * jj if jj > 0 else 0  # causal col narrowing
                        st2 = st_ps.tile([128, 2, 512], f32, tag="st")
                        nc.tensor.matmul(
                            st2[:, 0, cc0:512],
                            kv2_sb[0:64, j * 128:(j + 1) * 128],
                            qtr_sb[0:64, jt, c0 + cc0:c1],
                            start=True, stop=True,
                        )
                        nc.tensor.matmul(
                            st2[:, 1, cc0:512],
                            ko_sb[64:128, j * 128:(j + 1) * 128],
                            qtr_sb[64:128, jt, c0 + cc0:c1],
                            start=True, stop=True,
                        )
                        et2 = etp.tile([128, 2, 512], bf16, tag="et")
                        nc.scalar.activation(
                            et2[:, :, cc0:512], st2[:, :, cc0:512],
                            mybir.ActivationFunctionType.Exp)
                        if jj >= 0:
                            # triangular band: cols [cc0, cc0+128)
                            nc.vector.tensor_mul(
                                et2[:, 0, cc0:cc0 + 128], et2[:, 0, cc0:cc0 + 128], tril_sb)
                            nc.vector.tensor_mul(
                                et2[:, 1, cc0:cc0 + 128], et2[:, 1, cc0:cc0 + 128], tril_sb)
                        if debug_dump and si == 0 and jt == 0 and j == 0:
                            nc.sync.dma_start(out=dbg["et0"].ap(), in_=et2)
                        # per head: rows 0:64 = V out, row 64 = denominator
                        nc.tensor.matmul(
                            ut[:, 0, cc0:512], vext_sb[:, j, 0:65], et2[:, 0, cc0:512],
                            start=(j == 0), stop=(j == nblk - 1),
                        )
                        nc.tensor.matmul(
                            ut[:, 1, cc0:512], vext_sb[:, j, 0:65], et2[:, 1, cc0:512],
                            start=(j == 0), stop=(j == nblk - 1),
                        )
                    # normalize: at = v_out * (1/den). Broadcast the raw den
                    # rows to 64 partitions via a base-0 K=1 PE matmul, then
                    # reciprocal at partition base 0 and two muls that read
                    # the V rows straight from ut PSUM (one PSUM operand).
                    denb = rcp_p.tile([1, 2, 512], bf16, tag="denb")
                    nc.vector.tensor_copy(denb, ut[64:65, :, :])
                    db0 = qkv_ps.tile([64, 512], f32, tag="mm")
                    db1 = qkv_ps.tile([64, 512], f32, tag="mm")
                    nc.tensor.matmul(db0, ones_sb[0:1, 0:64], denb[0:1, 0, :],
                                     start=True, stop=True)
                    nc.tensor.matmul(db1, ones_sb[0:1, 0:64], denb[0:1, 1, :],
                                     start=True, stop=True)
                    rc64 = rcp_p.tile([64, 2, 512], f32, tag="rc64")
                    nc.vector.reciprocal_approx_fast(rc64[:, 0, :], db0)
                    nc.vector.reciprocal_approx_fast(rc64[:, 1, :], db1)
                    nc.vector.tensor_mul(at_sb[0:64, jt, c0:c1], ut[0:64, 0, :], rc64[:, 0, :])
                    tmp64 = rcp_p.tile([64, 512], bf16, tag="tmp64")
                    nc.vector.tensor_mul(tmp64, ut[0:64, 1, :], rc64[:, 1, :])
                    nc.gpsimd.dma_start(out=at_sb[64:128, jt, c0:c1], in_=tmp64)
                    if debug_dump and si == 0 and jt == 0:
                        nc.sync.dma_start(out=dbg["rc0"].ap(), in_=rc64)

            # ---- partial out-projection for superblock si ----
            def oproj(si):
                c0, c1 = si * 512, (si + 1) * 512
                ost = None
                for dt in range(DT):
                    po = qkv_ps.tile([128, 512], f32, tag="mm")
                    for jt in range(2):
                        nc.tensor.matmul(
                            po, wo_sb[:, jt, dt * 128:(dt + 1) * 128],
                            at_sb[:, jt, c0:c1],
                            start=(jt == 0), stop=(jt == 1),
                        )
                    if dt % 4 == 0:
                        ost = ostage_p.tile([128, 4, 512], f16, tag="ost")
                    if dt % 4 == 3:
                        nc.vector.tensor_copy(ost[:, dt % 4, :], po)
                    else:
                        nc.scalar.activation(ost[:, dt % 4, :], po, mybir.ActivationFunctionType.Copy)
                    if dt % 4 == 3:
                        g = dt - 3
                        nc.sync.dma_start(
                            out=out_r[:, g:g + 4, c0:c1], in_=ost)

            # ---- emission order == scheduler priority ----
            proj_chunk(0)
            att(0)
            proj_chunk(1)
            att(1)
            oproj(0)
            proj_chunk(2)
            att(2)
            oproj(1)
            proj_chunk(3)
            att(3)
            oproj(2)
            oproj(3)

            if debug_dump:
                nc.sync.dma_start(out=dbg["qtr"].ap(), in_=qtr_sb)
                nc.sync.dma_start(out=dbg["kv2"].ap(), in_=kv2_sb)
                nc.sync.dma_start(out=dbg["vext"].ap(), in_=vext_sb)
                nc.sync.dma_start(out=dbg["at"].ap(), in_=at_sb)

    nc.compile()
    return nc


_SIGMA = np.concatenate([np.arange(0, HD, 2), np.arange(1, HD, 2)])


def _prep_inputs(x, freqs_cis, wq, wk, wv, wo):
    """Host-side shard + layout prep. Returns per-core in_maps."""
    x = np.asarray(x, np.float32).reshape(S, D)
    freqs_cis = np.asarray(freqs_cis, np.float32)
    wq = np.asarray(wq, np.float32)
    wk = np.asarray(wk, np.float32)
    wv = np.asarray(wv, np.float32)
    wo = np.asarray(wo, np.float32)

    xt = np.ascontiguousarray(x.T).astype(BF16)

    cosT = np.ascontiguousarray(freqs_cis[:, :, 0].T)  # [32, S]
    sinT = np.ascontiguousarray(freqs_cis[:, :, 1].T)
    cosb = np.ascontiguousarray(np.tile(cosT, (4, 1))).astype(BF16)
    sinb = np.ascontiguousarray(
        np.concatenate([-sinT, sinT, -sinT, sinT], 0)).astype(BF16)

    tloc = np.arange(128)[:, None]
    cloc = np.arange(128)[None, :]
    tril = (tloc <= cloc).astype(np.float32).astype(BF16)
    ident64 = np.eye(64, dtype=np.float32).astype(BF16)

    scale = 1.0 / np.sqrt(HD)
    in_maps = []
    for g in range(NC):
        wqg = wq[g * R * HD:(g + 1) * R * HD].reshape(R, HD, D)[:, _SIGMA, :].reshape(R * HD, D)
        wq_tg = np.ascontiguousarray(wqg.T).astype(BF16)
        wkg = wk[g * HD:(g + 1) * HD][_SIGMA] * scale
        wvg = wv[g * HD:(g + 1) * HD]
        wkv_tg = np.ascontiguousarray(np.concatenate([wkg, wvg], 0).T).astype(BF16)
        wo_tg = np.ascontiguousarray(wo[:, g * R * HD:(g + 1) * R * HD].T).astype(BF16)
        in_maps.append({
            "xt": xt,
            "wq_t": wq_tg,
            "wkv_t": wkv_tg,
            "wo_t": wo_tg,
            "cosb": cosb,
            "sinb": sinb,
            "tril": tril,
            "ident64": ident64,
        })
    return in_maps


_CACHED = {}


def _get_program():
    if "nc" not in _CACHED:
        _CACHED["nc"] = _build_program()
    return _CACHED["nc"]


def kernel(x, freqs_cis, wq, wk, wv, wo, _trace=False):
    from concourse.bass_utils import run_bass_kernel_spmd

    nc = _get_program()
    in_maps = _prep_inputs(x, freqs_cis, wq, wk, wv, wo)
    res = run_bass_kernel_spmd(nc, in_maps, core_ids=list(range(NC)), trace=_trace)
    acc = np.zeros((D, S), np.float64)
    for c in range(NC):
        acc += res.results[c]["out_t"].astype(np.float64)
    out = np.ascontiguousarray(acc.T, dtype=np.float32).reshape(1, S, D)
    if _trace:
        return out, res
    return out


# revision 25
# speedup vs baseline: 1.0335x; 1.0192x over previous
"""Tensor-parallel Llama GQA attention layer (B=1, S=2048, D=2048, H=32, KV=8)
for 8 Trainium2 NeuronCores.

Sharding: one KV group per core (kv head g + its 4 q heads). Each core computes
its heads' attention and a partial out-projection (contraction over its 256
head-dim columns of wo); the host sums the 8 fp16 partials (the TP all-reduce)
and transposes back to [1, S, D].

On-core layout is feature-major (transposed): xt=[D,S], QT=[j,S], KT/VT=[hd,S].
Scores are built per (head-pair, s-superblock of 512, t-block of 128) as
ST=[t,s] tiles; softmax is unnormalized exp (scores are O(1) so no max
subtraction is needed) with the denominator obtained by ones columns in the
V stationary tile, and a single normalization divide at the end.

Schedule notes: emission order is the Tile scheduler's priority, so the
program interleaves projection chunks, attention superblocks and partial
out-projections (proj0, att0, proj1, att1, oproj0, proj2, att2, oproj1,
proj3, att3, oproj2, oproj3) to keep PE (the bottleneck engine) saturated
and at full p-state. Diagonal 128x512 score blocks are column-narrowed to
the causal region for MM1/exp/MM2, and the 0/1 mask multiply only touches
the 128-col triangular band (one shared tril tile). The softmax denominator
rows land inside the ut PSUM tile (ones col after V for the even head, before
V for the odd head, whose output sits at partitions 63:128), so normalization
is 2 reciprocals + a K=1 PE broadcast matmul + 2 partition-aligned DVE
multiplies - no partition-shift DMAs.
"""

import numpy as np
import ml_dtypes

S = 2048
D = 2048
H = 32
KV = 8
HD = 64
R = 4  # heads per kv group
NC = 8  # cores

BF16 = ml_dtypes.bfloat16
FP16 = np.float16


def _build_program(debug_dump=False):
    import concourse.mybir as mybir
    import concourse.tile as tile
    from concourse import bacc

    f32 = mybir.dt.float32
    f16 = mybir.dt.float16
    bf16 = mybir.dt.bfloat16

    nc = bacc.Bacc("TRN2", debug=False, num_devices=NC)
    dbg = {}
    if debug_dump:
        dbg["qtr"] = nc.dram_tensor("dbg_qtr", [128, 2, S], bf16, kind="ExternalOutput")
        dbg["kv2"] = nc.dram_tensor("dbg_kv2", [128, S], bf16, kind="ExternalOutput")
        dbg["vext"] = nc.dram_tensor("dbg_vext", [128, S // 128, 65], bf16, kind="ExternalOutput")
        dbg["at"] = nc.dram_tensor("dbg_at", [128, 2, S], bf16, kind="ExternalOutput")
        dbg["rc0"] = nc.dram_tensor("dbg_rc0", [64, 2, 512], f32, kind="ExternalOutput")
        dbg["et0"] = nc.dram_tensor("dbg_et0", [128, 2, 512], bf16, kind="ExternalOutput")

    xt = nc.dram_tensor("xt", [D, S], bf16, kind="ExternalInput")
    wq_t = nc.dram_tensor("wq_t", [D, R * HD], bf16, kind="ExternalInput")
    wkv_t = nc.dram_tensor("wkv_t", [D, 2 * HD], bf16, kind="ExternalInput")
    wo_t = nc.dram_tensor("wo_t", [R * HD, D], bf16, kind="ExternalInput")
    cosb = nc.dram_tensor("cosb", [128, S], bf16, kind="ExternalInput")
    sinb = nc.dram_tensor("sinb", [128, S], bf16, kind="ExternalInput")
    tril = nc.dram_tensor("tril", [128, 128], bf16, kind="ExternalInput")
    ident64 = nc.dram_tensor("ident64", [64, 64], bf16, kind="ExternalInput")
    # rope swap permutations as one-hot stationary matrices:
    # swapm = blockdiag(P64, P64); kswm = [[P64 | I64] , P64]
    swapm = nc.dram_tensor("swapm", [128, 128], bf16, kind="ExternalInput")
    kswm = nc.dram_tensor("kswm", [64, 192], bf16, kind="ExternalInput")
    out_t = nc.dram_tensor("out_t", [D, S], f16, kind="ExternalOutput")

    DT = D // 128  # 16 d tiles
    TT = S // 128  # 16 t blocks
    SB = S // 512  # 4 s superblocks

    with tile.TileContext(nc) as tc:
        with (
            tc.tile_pool(name="persist", bufs=1) as persist,
            tc.tile_pool(name="qstage", bufs=2) as qstage_p,
            tc.tile_pool(name="rtmp", bufs=2) as rtmp_p,
            tc.tile_pool(name="et", bufs=6) as etp,
            tc.tile_pool(name="rcp", bufs=2) as rcp_p,
            tc.tile_pool(name="ostage", bufs=3) as ostage_p,
            tc.tile_pool(name="st_ps", bufs=2, space="PSUM") as st_ps,
            tc.tile_pool(name="ut_ps", bufs=1, space="PSUM") as ut_ps,
            tc.tile_pool(name="qkv_ps", bufs=2, space="PSUM") as qkv_ps,
        ):
            # ---- persistent SBUF tensors ----
            xt_sb = persist.tile([128, DT, S], bf16)
            wq_sb = persist.tile([128, DT, R * HD], bf16)
            wkv_sb = persist.tile([128, DT, 2 * HD], bf16)
            wo_sb = persist.tile([128, 2, D], bf16)
            cos_sb = persist.tile([128, S], bf16)
            sin_sb = persist.tile([128, S], bf16)
            tril_sb = persist.tile([128, 128], bf16)
            ident_sb = persist.tile([128, 64], bf16)
            ones_sb = persist.tile([128, 64], bf16)
            swapm_sb = persist.tile([128, 128], bf16)
            kswm_sb = persist.tile([64, 192], bf16)

            qtr_sb = persist.tile([128, 2, S], bf16)   # roped Q, head-major
            kv2_sb = persist.tile([128, S], bf16)      # 0:64 roped K, 64:128 VT
            ko_sb = persist.tile([128, S], bf16)       # 64:128 roped K (odd heads)
            # V blocks [t, hd]: cols 0:64 = V, col 64 = ones (denominator)
            vext_sb = persist.tile([128, TT, 65], bf16)
            at_sb = persist.tile([128, 2, S], bf16)    # normalized attn out

            # ---- input DMA, ordered for the pipeline lead-in ----
            wkv_r = wkv_t.ap().rearrange("(dt p) j -> p dt j", p=128)
            wq_r = wq_t.ap().rearrange("(dt p) j -> p dt j", p=128)
            xt_r = xt.ap().rearrange("(dt p) s -> p dt s", p=128)
            out_r = out_t.ap().rearrange("(g p) s -> p g s", p=128)

            nc.sync.dma_start(out=wkv_sb, in_=wkv_r)
            nc.sync.dma_start(out=ident_sb[64:128, :], in_=ident64.ap())
            nc.sync.dma_start(out=swapm_sb, in_=swapm.ap())
            nc.sync.dma_start(out=kswm_sb, in_=kswm.ap())
            for dq in range(4):
                nc.sync.dma_start(out=xt_sb[:, 4 * dq:4 * dq + 4, 0:512],
                                  in_=xt_r[:, 4 * dq:4 * dq + 4, 0:512])
            nc.sync.dma_start(out=cos_sb[:, 0:512], in_=cosb.ap()[:, 0:512])
            nc.sync.dma_start(out=sin_sb[:, 0:512], in_=sinb.ap()[:, 0:512])
            nc.sync.dma_start(out=wq_sb, in_=wq_r)
            nc.sync.dma_start(out=tril_sb, in_=tril.ap())
            for sc in range(1, 4):
                c0, c1 = sc * 512, (sc + 1) * 512
                nc.sync.dma_start(out=xt_sb[:, :, c0:c1], in_=xt_r[:, :, c0:c1])
                nc.sync.dma_start(out=cos_sb[:, c0:c1], in_=cosb.ap()[:, c0:c1])
                nc.sync.dma_start(out=sin_sb[:, c0:c1], in_=sinb.ap()[:, c0:c1])
                if sc == 2:
                    for jt in range(2):
                        nc.sync.dma_start(out=wo_sb[:, jt, :], in_=wo_t.ap()[jt * 128:(jt + 1) * 128, :])

            nc.vector.memset(vext_sb, 1.0)  # ones col 64 stays
            nc.vector.memset(ones_sb, 1.0)

            kstage = qstage_p.tile([64, S], bf16, tag="kstage")
            qstage_tiles = [
                qstage_p.tile([128, S], bf16, tag="qstage", name=f"qst{i}")
                for i in range(2)
            ]

            # ---- RoPE: roped = x*C + swap(x)*S, swap done as a PE one-hot
            # matmul (no DMA on the critical path). C=[cos x4],
            # S=[-sin,+sin]x2; swap exchanges rows 0:32 <-> 32:64 per head.
            def rope_q(src, dst, c0, c1):
                sw = qkv_ps.tile([128, 512], f32, tag="mm")
                nc.tensor.matmul(sw, swapm_sb, src[:, c0:c1], start=True, stop=True)
                t1 = rtmp_p.tile([128, 512], bf16, tag="ropetmp")
                nc.vector.tensor_mul(t1, src[:, c0:c1], cos_sb[:, c0:c1])
                t2 = rtmp_p.tile([128, 512], bf16, tag="ropetmp2")
                nc.vector.tensor_mul(t2, sw, sin_sb[:, c0:c1])
                nc.vector.tensor_add(dst, t1, t2)

            # ---- projection chunk: KV proj + K rope + V transpose + Q proj ----
            def proj_chunk(si):
                c0, c1 = si * 512, (si + 1) * 512
                ps = qkv_ps.tile([128, 512], f32, tag="mm")
                for dt in range(DT):
                    nc.tensor.matmul(
                        ps, wkv_sb[:, dt, :], xt_sb[:, dt, c0:c1],
                        start=(dt == 0), stop=(dt == DT - 1),
                    )
                nc.vector.tensor_copy(kstage[:, c0:c1], ps[0:64, :])
                nc.vector.tensor_copy(kv2_sb[64:128, c0:c1], ps[64:128, :])
                # K rope at both PE strips: ps1 = [swap(K) ; K], ps2[64:] = swap(K)
                ps1 = qkv_ps.tile([128, 512], f32, tag="mm")
                nc.tensor.matmul(ps1, kswm_sb[:, 0:128], kstage[:, c0:c1], start=True, stop=True)
                ps2 = qkv_ps.tile([128, 512], f32, tag="mm")
                nc.tensor.matmul(ps2[64:128, :], kswm_sb[:, 128:192], kstage[:, c0:c1], start=True, stop=True)
                t1 = rtmp_p.tile([128, 512], bf16, tag="ropetmp")
                t2 = rtmp_p.tile([128, 512], bf16, tag="ropetmp2")
                nc.vector.tensor_mul(t1[0:64], kstage[:, c0:c1], cos_sb[0:64, c0:c1])
                nc.vector.tensor_mul(t2[0:64], ps1[0:64, :], sin_sb[0:64, c0:c1])
                nc.vector.tensor_add(kv2_sb[0:64, c0:c1], t1[0:64], t2[0:64])
                nc.vector.tensor_mul(t1[64:128], ps1[64:128, :], cos_sb[64:128, c0:c1])
                nc.vector.tensor_mul(t2[64:128], ps2[64:128, :], sin_sb[64:128, c0:c1])
                nc.vector.tensor_add(ko_sb[64:128, c0:c1], t1[64:128], t2[64:128])
                for tt in range(4 * si, 4 * si + 4):
                    vps = st_ps.tile([128, 64], bf16, tag="st")
                    nc.tensor.transpose(vps, kv2_sb[64:128, tt * 128:(tt + 1) * 128], ident_sb[64:128, :])
                    nc.vector.tensor_copy(vext_sb[:, tt, 0:64], vps)
                for jt in range(2):
                    qst = qstage_tiles[jt]
                    ps = qkv_ps.tile([128, 512], f32, tag="mm")
                    for dt in range(DT):
                        nc.tensor.matmul(
                            ps, wq_sb[:, dt, jt * 128:(jt + 1) * 128], xt_sb[:, dt, c0:c1],
                            start=(dt == 0), stop=(dt == DT - 1),
                        )
                    nc.vector.tensor_copy(qst[:, c0:c1], ps)
                    rope_q(qst, qtr_sb[:, jt, c0:c1], c0, c1)

            # ---- attention superblock: heads in pairs (2jt, 2jt+1) ----
            # even head scores at PE contraction rows 0:64, odd at 64:128 ->
            # MM1s overlap in disjoint row strips.
            def att(si):
                nblk = 4 * (si + 1)
                c0, c1 = si * 512, (si + 1) * 512
                for jt in range(2):
                    ut = ut_ps.tile([65, 2, 512], f32, tag="ut")
                    for j in range(nblk):
                        jj = j - 4 * si  # >=0 on the diagonal superblock
                        cc0 = 128 * jj if jj > 0 else 0  # causal col narrowing
                        st2 = st_ps.tile([128, 2, 512], f32, tag="st")
                        nc.tensor.matmul(
                            st2[:, 0, cc0:512],
                            kv2_sb[0:64, j * 128:(j + 1) * 128],
                            qtr_sb[0:64, jt, c0 + cc0:c1],
                            start=True, stop=True,
                        )
                        nc.tensor.matmul(
                            st2[:, 1, cc0:512],
                            ko_sb[64:128, j * 128:(j + 1) * 128],
                            qtr_sb[64:128, jt, c0 + cc0:c1],
                            start=True, stop=True,
                        )
                        et2 = etp.tile([128, 2, 512], bf16, tag="et")
                        nc.scalar.activation(
                            et2[:, :, cc0:512], st2[:, :, cc0:512],
                            mybir.ActivationFunctionType.Exp)
                        if jj >= 0:
                            # triangular band: cols [cc0, cc0+128)
                            nc.vector.tensor_mul(
                                et2[:, 0, cc0:cc0 + 128], et2[:, 0, cc0:cc0 + 128], tril_sb)
                            nc.vector.tensor_mul(
                                et2[:, 1, cc0:cc0 + 128], et2[:, 1, cc0:cc0 + 128], tril_sb)
                        if debug_dump and si == 0 and jt == 0 and j == 0:
                            nc.sync.dma_start(out=dbg["et0"].ap(), in_=et2)
                        # per head: rows 0:64 = V out, row 64 = denominator
                        nc.tensor.matmul(
                            ut[:, 0, cc0:512], vext_sb[:, j, 0:65], et2[:, 0, cc0:512],
                            start=(j == 0), stop=(j == nblk - 1),
                        )
                        nc.tensor.matmul(
                            ut[:, 1, cc0:512], vext_sb[:, j, 0:65], et2[:, 1, cc0:512],
                            start=(j == 0), stop=(j == nblk - 1),
                        )
                    # normalize: at = v_out * (1/den). Broadcast the raw den
                    # rows to 64 partitions via a base-0 K=1 PE matmul, then
                    # reciprocal at partition base 0 and two muls that read
                    # the V rows straight from ut PSUM (one PSUM operand).
                    denb = rcp_p.tile([1, 2, 512], bf16, tag="denb")
                    nc.vector.tensor_copy(denb, ut[64:65, :, :])
                    db0 = qkv_ps.tile([64, 512], f32, tag="mm")
                    db1 = qkv_ps.tile([64, 512], f32, tag="mm")
                    nc.tensor.matmul(db0, ones_sb[0:1, 0:64], denb[0:1, 0, :],
                                     start=True, stop=True)
                    nc.tensor.matmul(db1, ones_sb[0:1, 0:64], denb[0:1, 1, :],
                                     start=True, stop=True)
                    rc64 = rcp_p.tile([64, 2, 512], f32, tag="rc64")
                    nc.vector.reciprocal_approx_fast(rc64[:, 0, :], db0)
                    nc.vector.reciprocal_approx_fast(rc64[:, 1, :], db1)
                    nc.vector.tensor_mul(at_sb[0:64, jt, c0:c1], ut[0:64, 0, :], rc64[:, 0, :])
                    tmp64 = rcp_p.tile([64, 512], bf16, tag="tmp64")
                    nc.vector.tensor_mul(tmp64, ut[0:64, 1, :], rc64[:, 1, :])
                    nc.gpsimd.dma_start(out=at_sb[64:128, jt, c0:c1], in_=tmp64)
                    if debug_dump and si == 0 and jt == 0:
                        nc.sync.dma_start(out=dbg["rc0"].ap(), in_=rc64)

            # ---- partial out-projection for superblock si ----
            def oproj(si):
                c0, c1 = si * 512, (si + 1) * 512
                gsz = 2 if si == SB - 1 else 4  # finer tail granularity
                ost = None
                for dt in range(DT):
                    po = qkv_ps.tile([128, 512], f32, tag="mm")
                    for jt in range(2):
                        nc.tensor.matmul(
                            po, wo_sb[:, jt, dt * 128:(dt + 1) * 128],
                            at_sb[:, jt, c0:c1],
                            start=(jt == 0), stop=(jt == 1),
                        )
                    if dt % gsz == 0:
                        ost = ostage_p.tile([128, gsz, 512], f16, tag="ost")
                    if dt % gsz == gsz - 1:
                        nc.vector.tensor_copy(ost[:, dt % gsz, :], po)
                    else:
                        nc.scalar.activation(ost[:, dt % gsz, :], po, mybir.ActivationFunctionType.Copy)
                    if dt % gsz == gsz - 1:
                        g = dt - gsz + 1
                        nc.sync.dma_start(
                            out=out_r[:, g:g + gsz, c0:c1], in_=ost)

            # ---- emission order == scheduler priority ----
            proj_chunk(0)
            att(0)
            proj_chunk(1)
            att(1)
            oproj(0)
            proj_chunk(2)
            att(2)
            oproj(1)
            proj_chunk(3)
            att(3)
            oproj(2)
            oproj(3)

            if debug_dump:
                nc.sync.dma_start(out=dbg["qtr"].ap(), in_=qtr_sb)
                nc.sync.dma_start(out=dbg["kv2"].ap(), in_=kv2_sb)
                nc.sync.dma_start(out=dbg["vext"].ap(), in_=vext_sb)
                nc.sync.dma_start(out=dbg["at"].ap(), in_=at_sb)

    nc.compile()
    return nc


_SIGMA = np.concatenate([np.arange(0, HD, 2), np.arange(1, HD, 2)])


def _prep_inputs(x, freqs_cis, wq, wk, wv, wo):
    """Host-side shard + layout prep. Returns per-core in_maps."""
    x = np.asarray(x, np.float32).reshape(S, D)
    freqs_cis = np.asarray(freqs_cis, np.float32)
    wq = np.asarray(wq, np.float32)
    wk = np.asarray(wk, np.float32)
    wv = np.asarray(wv, np.float32)
    wo = np.asarray(wo, np.float32)

    xt = np.ascontiguousarray(x.T).astype(BF16)

    cosT = np.ascontiguousarray(freqs_cis[:, :, 0].T)  # [32, S]
    sinT = np.ascontiguousarray(freqs_cis[:, :, 1].T)
    cosb = np.ascontiguousarray(np.tile(cosT, (4, 1))).astype(BF16)
    sinb = np.ascontiguousarray(
        np.concatenate([-sinT, sinT, -sinT, sinT], 0)).astype(BF16)

    tloc = np.arange(128)[:, None]
    cloc = np.arange(128)[None, :]
    tril = (tloc <= cloc).astype(np.float32).astype(BF16)
    ident64 = np.eye(64, dtype=np.float32).astype(BF16)

    # one-hot swap matrices for the PE-matmul rope: P64[k, m] = 1 iff
    # m = (k+32) % 64 (self-inverse pair swap within a 64-row head block)
    P64 = np.zeros((64, 64), np.float32)
    P64[np.arange(64), (np.arange(64) + 32) % 64] = 1.0
    I64 = np.eye(64, dtype=np.float32)
    Z64 = np.zeros((64, 64), np.float32)
    swapm = np.block([[P64, Z64], [Z64, P64]]).astype(BF16)  # [128, 128]
    kswm = np.concatenate([P64, I64, P64], axis=1).astype(BF16)  # [64, 192]

    scale = 1.0 / np.sqrt(HD)
    in_maps = []
    for g in range(NC):
        wqg = wq[g * R * HD:(g + 1) * R * HD].reshape(R, HD, D)[:, _SIGMA, :].reshape(R * HD, D)
        wq_tg = np.ascontiguousarray(wqg.T).astype(BF16)
        wkg = wk[g * HD:(g + 1) * HD][_SIGMA] * scale
        wvg = wv[g * HD:(g + 1) * HD]
        wkv_tg = np.ascontiguousarray(np.concatenate([wkg, wvg], 0).T).astype(BF16)
        wo_tg = np.ascontiguousarray(wo[:, g * R * HD:(g + 1) * R * HD].T).astype(BF16)
        in_maps.append({
            "xt": xt,
            "wq_t": wq_tg,
            "wkv_t": wkv_tg,
            "wo_t": wo_tg,
            "cosb": cosb,
            "sinb": sinb,
            "tril": tril,
            "ident64": ident64,
            "swapm": swapm,
            "kswm": kswm,
        })
    return in_maps


_CACHED = {}


def _get_program():
    if "nc" not in _CACHED:
        _CACHED["nc"] = _build_program()
    return _CACHED["nc"]


def kernel(x, freqs_cis, wq, wk, wv, wo, _trace=False):
    from concourse.bass_utils import run_bass_kernel_spmd

    nc = _get_program()
    in_maps = _prep_inputs(x, freqs_cis, wq, wk, wv, wo)
    res = run_bass_kernel_spmd(nc, in_maps, core_ids=list(range(NC)), trace=_trace)
    acc = np.zeros((D, S), np.float64)
    for c in range(NC):
        acc += res.results[c]["out_t"].astype(np.float64)
    out = np.ascontiguousarray(acc.T, dtype=np.float32).reshape(1, S, D)
    if _trace:
        return out, res
    return out


# revision 31
# speedup vs baseline: 1.0820x; 1.0469x over previous
"""Tensor-parallel Llama GQA attention layer (B=1, S=2048, D=2048, H=32, KV=8)
for 8 Trainium2 NeuronCores.

Sharding: one KV group per core (kv head g + its 4 q heads). Each core computes
its heads' attention and a partial out-projection (contraction over its 256
head-dim columns of wo); the host sums the 8 fp16 partials (the TP all-reduce)
and transposes back to [1, S, D].

On-core layout is feature-major (transposed): xt=[D,S], QT=[j,S], KT/VT=[hd,S].
Scores are built per (head-pair, s-superblock of 512, t-block of 128) as
ST=[t,s] tiles; softmax is unnormalized exp (scores are O(1) so no max
subtraction is needed) with the denominator obtained by ones columns in the
V stationary tile, and a single normalization divide at the end.

Schedule notes: emission order is the Tile scheduler's priority, so the
program interleaves projection chunks, attention superblocks and partial
out-projections (proj0, att0, proj1, att1, oproj0, proj2, att2, oproj1,
proj3, att3, oproj2, oproj3) to keep PE (the bottleneck engine) saturated
and at full p-state. Diagonal 128x512 score blocks are column-narrowed to
the causal region for MM1/exp/MM2, and the 0/1 mask multiply only touches
the 128-col triangular band (one shared tril tile). The softmax denominator
rows land inside the ut PSUM tile (ones col after V for the even head, before
V for the odd head, whose output sits at partitions 63:128), so normalization
is 2 reciprocals + a K=1 PE broadcast matmul + 2 partition-aligned DVE
multiplies - no partition-shift DMAs.
"""

import numpy as np
import ml_dtypes

S = 2048
D = 2048
H = 32
KV = 8
HD = 64
R = 4  # heads per kv group
NC = 8  # cores

BF16 = ml_dtypes.bfloat16
FP16 = np.float16


def _build_program(debug_dump=False):
    import concourse.mybir as mybir
    import concourse.tile as tile
    from concourse import bacc

    f32 = mybir.dt.float32
    f16 = mybir.dt.float16
    bf16 = mybir.dt.bfloat16

    nc = bacc.Bacc("TRN2", debug=False, num_devices=NC)
    dbg = {}
    if debug_dump:
        dbg["qtr"] = nc.dram_tensor("dbg_qtr", [128, 2, S], bf16, kind="ExternalOutput")
        dbg["kv2"] = nc.dram_tensor("dbg_kv2", [128, S], bf16, kind="ExternalOutput")
        dbg["vext"] = nc.dram_tensor("dbg_vext", [128, S // 128, 65], bf16, kind="ExternalOutput")
        dbg["at"] = nc.dram_tensor("dbg_at", [128, 2, S], bf16, kind="ExternalOutput")
        dbg["rc0"] = nc.dram_tensor("dbg_rc0", [64, 2, 512], f32, kind="ExternalOutput")
        dbg["et0"] = nc.dram_tensor("dbg_et0", [128, 2, 512], bf16, kind="ExternalOutput")

    xt = nc.dram_tensor("xt", [D, S], bf16, kind="ExternalInput")
    wq_t = nc.dram_tensor("wq_t", [D, R * HD], bf16, kind="ExternalInput")
    wkv_t = nc.dram_tensor("wkv_t", [D, 2 * HD], bf16, kind="ExternalInput")
    wo_t = nc.dram_tensor("wo_t", [R * HD, D], bf16, kind="ExternalInput")
    cosb = nc.dram_tensor("cosb", [128, S], bf16, kind="ExternalInput")
    sinb = nc.dram_tensor("sinb", [128, S], bf16, kind="ExternalInput")
    tril = nc.dram_tensor("tril", [128, 128], bf16, kind="ExternalInput")
    ident64 = nc.dram_tensor("ident64", [64, 64], bf16, kind="ExternalInput")
    # rope swap permutations as one-hot stationary matrices:
    # swapm = blockdiag(P64, P64); kswm = [[P64 | I64] , P64]
    swapm = nc.dram_tensor("swapm", [128, 128], bf16, kind="ExternalInput")
    kswm = nc.dram_tensor("kswm", [64, 192], bf16, kind="ExternalInput")
    out_t = nc.dram_tensor("out_t", [D, S], f16, kind="ExternalOutput")

    DT = D // 128  # 16 d tiles
    TT = S // 128  # 16 t blocks
    SB = S // 512  # 4 s superblocks

    with tile.TileContext(nc) as tc:
        with (
            tc.tile_pool(name="persist", bufs=1) as persist,
            tc.tile_pool(name="qstage", bufs=2) as qstage_p,
            tc.tile_pool(name="rtmp", bufs=2) as rtmp_p,
            tc.tile_pool(name="et", bufs=6) as etp,
            tc.tile_pool(name="rcp", bufs=2) as rcp_p,
            tc.tile_pool(name="ostage", bufs=3) as ostage_p,
            tc.tile_pool(name="st_ps", bufs=2, space="PSUM") as st_ps,
            tc.tile_pool(name="ut_ps", bufs=1, space="PSUM") as ut_ps,
            tc.tile_pool(name="qkv_ps", bufs=2, space="PSUM") as qkv_ps,
        ):
            # ---- persistent SBUF tensors ----
            xt_sb = persist.tile([128, DT, S], bf16)
            wq_sb = persist.tile([128, DT, R * HD], bf16)
            wkv_sb = persist.tile([128, DT, 2 * HD], bf16)
            wo_sb = persist.tile([128, 2, D], bf16)
            cos_sb = persist.tile([128, S], bf16)
            sin_sb = persist.tile([128, S], bf16)
            tril_sb = persist.tile([128, 128], bf16)
            ident_sb = persist.tile([128, 64], bf16)
            ones_sb = persist.tile([128, 64], bf16)
            swapm_sb = persist.tile([128, 128], bf16)
            kswm_sb = persist.tile([64, 192], bf16)

            qtr_sb = persist.tile([128, 2, S], bf16)   # roped Q, head-major
            kv2_sb = persist.tile([128, S], bf16)      # 0:64 roped K, 64:128 VT
            ko_sb = persist.tile([128, S], bf16)       # 64:128 roped K (odd heads)
            # V blocks [t, hd]: cols 0:64 = V, col 64 = ones (denominator)
            vext_sb = persist.tile([128, TT, 65], bf16)
            at_sb = persist.tile([128, 2, S], bf16)    # normalized attn out

            # ---- input DMA, ordered for the pipeline lead-in ----
            wkv_r = wkv_t.ap().rearrange("(dt p) j -> p dt j", p=128)
            wq_r = wq_t.ap().rearrange("(dt p) j -> p dt j", p=128)
            xt_r = xt.ap().rearrange("(dt p) s -> p dt s", p=128)
            out_r = out_t.ap().rearrange("(g p) s -> p g s", p=128)

            nc.sync.dma_start(out=wkv_sb, in_=wkv_r)
            nc.sync.dma_start(out=ident_sb[64:128, :], in_=ident64.ap())
            nc.sync.dma_start(out=swapm_sb, in_=swapm.ap())
            nc.sync.dma_start(out=kswm_sb, in_=kswm.ap())
            for dq in range(4):
                nc.sync.dma_start(out=xt_sb[:, 4 * dq:4 * dq + 4, 0:512],
                                  in_=xt_r[:, 4 * dq:4 * dq + 4, 0:512])
            nc.sync.dma_start(out=cos_sb[:, 0:512], in_=cosb.ap()[:, 0:512])
            nc.sync.dma_start(out=sin_sb[:, 0:512], in_=sinb.ap()[:, 0:512])
            nc.sync.dma_start(out=wq_sb, in_=wq_r)
            nc.sync.dma_start(out=tril_sb, in_=tril.ap())
            for sc in range(1, 4):
                c0, c1 = sc * 512, (sc + 1) * 512
                nc.sync.dma_start(out=xt_sb[:, :, c0:c1], in_=xt_r[:, :, c0:c1])
                nc.sync.dma_start(out=cos_sb[:, c0:c1], in_=cosb.ap()[:, c0:c1])
                nc.sync.dma_start(out=sin_sb[:, c0:c1], in_=sinb.ap()[:, c0:c1])
                if sc == 2:
                    for jt in range(2):
                        nc.sync.dma_start(out=wo_sb[:, jt, :], in_=wo_t.ap()[jt * 128:(jt + 1) * 128, :])

            nc.vector.memset(vext_sb, 1.0)  # ones col 64 stays
            nc.vector.memset(ones_sb, 1.0)

            kstage = qstage_p.tile([64, S], bf16, tag="kstage")
            qstage_tiles = [
                qstage_p.tile([128, S], bf16, tag="qstage", name=f"qst{i}")
                for i in range(2)
            ]

            # ---- RoPE: roped = x*C + swap(x)*S, swap done as a PE one-hot
            # matmul (no DMA on the critical path). C=[cos x4],
            # S=[-sin,+sin]x2; swap exchanges rows 0:32 <-> 32:64 per head.
            def rope_q(src, dst, c0, c1):
                sw = qkv_ps.tile([128, 512], f32, tag="mm")
                nc.tensor.matmul(sw, swapm_sb, src[:, c0:c1], start=True, stop=True)
                t1 = rtmp_p.tile([128, 512], bf16, tag="ropetmp")
                nc.vector.tensor_mul(t1, src[:, c0:c1], cos_sb[:, c0:c1])
                t2 = rtmp_p.tile([128, 512], bf16, tag="ropetmp2")
                nc.vector.tensor_mul(t2, sw, sin_sb[:, c0:c1])
                nc.vector.tensor_add(dst, t1, t2)

            # ---- projection chunk: KV proj + K rope + V transpose + Q proj ----
            def proj_chunk(si):
                c0, c1 = si * 512, (si + 1) * 512
                ps = qkv_ps.tile([128, 512], f32, tag="mm")
                for dt in range(DT):
                    nc.tensor.matmul(
                        ps, wkv_sb[:, dt, :], xt_sb[:, dt, c0:c1],
                        start=(dt == 0), stop=(dt == DT - 1),
                    )
                nc.vector.tensor_copy(kstage[:, c0:c1], ps[0:64, :])
                nc.vector.tensor_copy(kv2_sb[64:128, c0:c1], ps[64:128, :])
                # K rope at both PE strips: ps1 = [swap(K) ; K], ps2[64:] = swap(K)
                ps1 = qkv_ps.tile([128, 512], f32, tag="mm")
                nc.tensor.matmul(ps1, kswm_sb[:, 0:128], kstage[:, c0:c1], start=True, stop=True)
                ps2 = qkv_ps.tile([128, 512], f32, tag="mm")
                nc.tensor.matmul(ps2[64:128, :], kswm_sb[:, 128:192], kstage[:, c0:c1], start=True, stop=True)
                t1 = rtmp_p.tile([128, 512], bf16, tag="ropetmp")
                t2 = rtmp_p.tile([128, 512], bf16, tag="ropetmp2")
                nc.vector.tensor_mul(t1[0:64], kstage[:, c0:c1], cos_sb[0:64, c0:c1])
                nc.vector.tensor_mul(t2[0:64], ps1[0:64, :], sin_sb[0:64, c0:c1])
                nc.vector.tensor_add(kv2_sb[0:64, c0:c1], t1[0:64], t2[0:64])
                nc.vector.tensor_mul(t1[64:128], ps1[64:128, :], cos_sb[64:128, c0:c1])
                nc.vector.tensor_mul(t2[64:128], ps2[64:128, :], sin_sb[64:128, c0:c1])
                nc.vector.tensor_add(ko_sb[64:128, c0:c1], t1[64:128], t2[64:128])
                for tt in range(4 * si, 4 * si + 4):
                    vps = st_ps.tile([128, 64], bf16, tag="st")
                    nc.tensor.transpose(vps, kv2_sb[64:128, tt * 128:(tt + 1) * 128], ident_sb[64:128, :])
                    nc.vector.tensor_copy(vext_sb[:, tt, 0:64], vps)
                for jt in range(2):
                    qst = qstage_tiles[jt]
                    ps = qkv_ps.tile([128, 512], f32, tag="mm")
                    for dt in range(DT):
                        nc.tensor.matmul(
                            ps, wq_sb[:, dt, jt * 128:(jt + 1) * 128], xt_sb[:, dt, c0:c1],
                            start=(dt == 0), stop=(dt == DT - 1),
                        )
                    nc.vector.tensor_copy(qst[:, c0:c1], ps)
                    rope_q(qst, qtr_sb[:, jt, c0:c1], c0, c1)

            # ---- attention superblock: heads in pairs (2jt, 2jt+1) ----
            # even head scores at PE contraction rows 0:64, odd at 64:128 ->
            # MM1s overlap in disjoint row strips.
            def att(si):
                nblk = 4 * (si + 1)
                c0, c1 = si * 512, (si + 1) * 512
                for jt in range(2):
                    ut = ut_ps.tile([65, 2, 512], f32, tag="ut")
                    for j in range(nblk):
                        jj = j - 4 * si  # >=0 on the diagonal superblock
                        cc0 = 128 * jj if jj > 0 else 0  # causal col narrowing
                        st2 = st_ps.tile([128, 2, 512], f32, tag="st")
                        nc.tensor.matmul(
                            st2[:, 0, cc0:512],
                            kv2_sb[0:64, j * 128:(j + 1) * 128],
                            qtr_sb[0:64, jt, c0 + cc0:c1],
                            start=True, stop=True,
                        )
                        nc.tensor.matmul(
                            st2[:, 1, cc0:512],
                            ko_sb[64:128, j * 128:(j + 1) * 128],
                            qtr_sb[64:128, jt, c0 + cc0:c1],
                            start=True, stop=True,
                        )
                        et2 = etp.tile([128, 2, 512], bf16, tag="et")
                        nc.scalar.activation(
                            et2[:, :, cc0:512], st2[:, :, cc0:512],
                            mybir.ActivationFunctionType.Exp)
                        if jj >= 0:
                            # triangular band: cols [cc0, cc0+128)
                            nc.vector.tensor_mul(
                                et2[:, 0, cc0:cc0 + 128], et2[:, 0, cc0:cc0 + 128], tril_sb)
                            nc.vector.tensor_mul(
                                et2[:, 1, cc0:cc0 + 128], et2[:, 1, cc0:cc0 + 128], tril_sb)
                        if debug_dump and si == 0 and jt == 0 and j == 0:
                            nc.sync.dma_start(out=dbg["et0"].ap(), in_=et2)
                        # per head: rows 0:64 = V out, row 64 = denominator
                        nc.tensor.matmul(
                            ut[:, 0, cc0:512], vext_sb[:, j, 0:65], et2[:, 0, cc0:512],
                            start=(j == 0), stop=(j == nblk - 1),
                        )
                        nc.tensor.matmul(
                            ut[:, 1, cc0:512], vext_sb[:, j, 0:65], et2[:, 1, cc0:512],
                            start=(j == 0), stop=(j == nblk - 1),
                        )
                    # normalize: at = v_out * (1/den). Evacuate ut whole (so
                    # the PSUM slot frees after one copy, not after the full
                    # chain), broadcast the raw den rows via base-0 K=1 PE
                    # matmuls, reciprocal at base 0, two aligned muls.
                    utsb = rcp_p.tile([65, 2, 512], f32, tag="utsb")
                    nc.vector.tensor_copy(utsb, ut)
                    denb = rcp_p.tile([1, 2, 512], bf16, tag="denb")
                    nc.vector.tensor_copy(denb, utsb[64:65, :, :])
                    db0 = qkv_ps.tile([64, 512], f32, tag="mm")
                    db1 = qkv_ps.tile([64, 512], f32, tag="mm")
                    nc.tensor.matmul(db0, ones_sb[0:1, 0:64], denb[0:1, 0, :],
                                     start=True, stop=True)
                    nc.tensor.matmul(db1, ones_sb[0:1, 0:64], denb[0:1, 1, :],
                                     start=True, stop=True)
                    rc64 = rcp_p.tile([64, 2, 512], f32, tag="rc64")
                    nc.vector.reciprocal_approx_fast(rc64[:, 0, :], db0)
                    nc.vector.reciprocal_approx_fast(rc64[:, 1, :], db1)
                    nc.vector.tensor_mul(at_sb[0:64, jt, c0:c1], utsb[0:64, 0, :], rc64[:, 0, :])
                    tmp64 = rcp_p.tile([64, 512], bf16, tag="tmp64")
                    nc.vector.tensor_mul(tmp64, utsb[0:64, 1, :], rc64[:, 1, :])
                    nc.gpsimd.dma_start(out=at_sb[64:128, jt, c0:c1], in_=tmp64)
                    if debug_dump and si == 0 and jt == 0:
                        nc.sync.dma_start(out=dbg["rc0"].ap(), in_=rc64)

            # ---- partial out-projection for superblock si ----
            def oproj(si):
                c0, c1 = si * 512, (si + 1) * 512
                gsz = 2 if si == SB - 1 else 4  # finer tail granularity
                ost = None
                for dt in range(DT):
                    po = qkv_ps.tile([128, 512], f32, tag="mm")
                    for jt in range(2):
                        nc.tensor.matmul(
                            po, wo_sb[:, jt, dt * 128:(dt + 1) * 128],
                            at_sb[:, jt, c0:c1],
                            start=(jt == 0), stop=(jt == 1),
                        )
                    if dt % gsz == 0:
                        ost = ostage_p.tile([128, gsz, 512], f16, tag="ost")
                    if dt % gsz == gsz - 1:
                        nc.vector.tensor_copy(ost[:, dt % gsz, :], po)
                    else:
                        nc.scalar.activation(ost[:, dt % gsz, :], po, mybir.ActivationFunctionType.Copy)
                    if dt % gsz == gsz - 1:
                        g = dt - gsz + 1
                        nc.sync.dma_start(
                            out=out_r[:, g:g + gsz, c0:c1], in_=ost)

            # ---- emission order == scheduler priority ----
            proj_chunk(0)
            proj_chunk(1)
            att(0)
            proj_chunk(2)
            att(1)
            oproj(0)
            proj_chunk(3)
            att(2)
            oproj(1)
            att(3)
            oproj(2)
            oproj(3)

            if debug_dump:
                nc.sync.dma_start(out=dbg["qtr"].ap(), in_=qtr_sb)
                nc.sync.dma_start(out=dbg["kv2"].ap(), in_=kv2_sb)
                nc.sync.dma_start(out=dbg["vext"].ap(), in_=vext_sb)
                nc.sync.dma_start(out=dbg["at"].ap(), in_=at_sb)

    nc.compile()
    return nc


_SIGMA = np.concatenate([np.arange(0, HD, 2), np.arange(1, HD, 2)])


def _prep_inputs(x, freqs_cis, wq, wk, wv, wo):
    """Host-side shard + layout prep. Returns per-core in_maps."""
    x = np.asarray(x, np.float32).reshape(S, D)
    freqs_cis = np.asarray(freqs_cis, np.float32)
    wq = np.asarray(wq, np.float32)
    wk = np.asarray(wk, np.float32)
    wv = np.asarray(wv, np.float32)
    wo = np.asarray(wo, np.float32)

    xt = np.ascontiguousarray(x.T).astype(BF16)

    cosT = np.ascontiguousarray(freqs_cis[:, :, 0].T)  # [32, S]
    sinT = np.ascontiguousarray(freqs_cis[:, :, 1].T)
    cosb = np.ascontiguousarray(np.tile(cosT, (4, 1))).astype(BF16)
    sinb = np.ascontiguousarray(
        np.concatenate([-sinT, sinT, -sinT, sinT], 0)).astype(BF16)

    tloc = np.arange(128)[:, None]
    cloc = np.arange(128)[None, :]
    tril = (tloc <= cloc).astype(np.float32).astype(BF16)
    ident64 = np.eye(64, dtype=np.float32).astype(BF16)

    # one-hot swap matrices for the PE-matmul rope: P64[k, m] = 1 iff
    # m = (k+32) % 64 (self-inverse pair swap within a 64-row head block)
    P64 = np.zeros((64, 64), np.float32)
    P64[np.arange(64), (np.arange(64) + 32) % 64] = 1.0
    I64 = np.eye(64, dtype=np.float32)
    Z64 = np.zeros((64, 64), np.float32)
    swapm = np.block([[P64, Z64], [Z64, P64]]).astype(BF16)  # [128, 128]
    kswm = np.concatenate([P64, I64, P64], axis=1).astype(BF16)  # [64, 192]

    scale = 1.0 / np.sqrt(HD)
    in_maps = []
    for g in range(NC):
        wqg = wq[g * R * HD:(g + 1) * R * HD].reshape(R, HD, D)[:, _SIGMA, :].reshape(R * HD, D)
        wq_tg = np.ascontiguousarray(wqg.T).astype(BF16)
        wkg = wk[g * HD:(g + 1) * HD][_SIGMA] * scale
        wvg = wv[g * HD:(g + 1) * HD]
        wkv_tg = np.ascontiguousarray(np.concatenate([wkg, wvg], 0).T).astype(BF16)
        wo_tg = np.ascontiguousarray(wo[:, g * R * HD:(g + 1) * R * HD].T).astype(BF16)
        in_maps.append({
            "xt": xt,
            "wq_t": wq_tg,
            "wkv_t": wkv_tg,
            "wo_t": wo_tg,
            "cosb": cosb,
            "sinb": sinb,
            "tril": tril,
            "ident64": ident64,
            "swapm": swapm,
            "kswm": kswm,
        })
    return in_maps


_CACHED = {}


def _get_program():
    if "nc" not in _CACHED:
        _CACHED["nc"] = _build_program()
    return _CACHED["nc"]


def kernel(x, freqs_cis, wq, wk, wv, wo, _trace=False):
    from concourse.bass_utils import run_bass_kernel_spmd

    nc = _get_program()
    in_maps = _prep_inputs(x, freqs_cis, wq, wk, wv, wo)
    res = run_bass_kernel_spmd(nc, in_maps, core_ids=list(range(NC)), trace=_trace)
    acc = np.zeros((D, S), np.float64)
    for c in range(NC):
        acc += res.results[c]["out_t"].astype(np.float64)
    out = np.ascontiguousarray(acc.T, dtype=np.float32).reshape(1, S, D)
    if _trace:
        return out, res
    return out


# revision 33
# speedup vs baseline: 1.1095x; 1.0254x over previous
"""Tensor-parallel Llama GQA attention layer (B=1, S=2048, D=2048, H=32, KV=8)
for 8 Trainium2 NeuronCores.

Sharding: one KV group per core (kv head g + its 4 q heads). Each core computes
its heads' attention and a partial out-projection (contraction over its 256
head-dim columns of wo); the host sums the 8 fp16 partials (the TP all-reduce)
and transposes back to [1, S, D].

On-core layout is feature-major (transposed): xt=[D,S], QT=[j,S], KT/VT=[hd,S].
Scores are built per (head-pair, s-superblock of 512, t-block of 128) as
ST=[t,s] tiles; softmax is unnormalized exp (scores are O(1) so no max
subtraction is needed) with the denominator obtained by ones columns in the
V stationary tile, and a single normalization divide at the end.

Schedule notes: emission order is the Tile scheduler's priority, so the
program interleaves projection chunks, attention superblocks and partial
out-projections (proj0, att0, proj1, att1, oproj0, proj2, att2, oproj1,
proj3, att3, oproj2, oproj3) to keep PE (the bottleneck engine) saturated
and at full p-state. Diagonal 128x512 score blocks are column-narrowed to
the causal region for MM1/exp/MM2, and the 0/1 mask multiply only touches
the 128-col triangular band (one shared tril tile). The softmax denominator
rows land inside the ut PSUM tile (ones col after V for the even head, before
V for the odd head, whose output sits at partitions 63:128), so normalization
is 2 reciprocals + a K=1 PE broadcast matmul + 2 partition-aligned DVE
multiplies - no partition-shift DMAs.
"""

import numpy as np
import ml_dtypes

S = 2048
D = 2048
H = 32
KV = 8
HD = 64
R = 4  # heads per kv group
NC = 8  # cores

BF16 = ml_dtypes.bfloat16
FP16 = np.float16


def _build_program(debug_dump=False):
    import concourse.mybir as mybir
    import concourse.tile as tile
    from concourse import bacc

    f32 = mybir.dt.float32
    f16 = mybir.dt.float16
    bf16 = mybir.dt.bfloat16

    nc = bacc.Bacc("TRN2", debug=False, num_devices=NC)
    dbg = {}
    if debug_dump:
        dbg["qtr"] = nc.dram_tensor("dbg_qtr", [128, 2, S], bf16, kind="ExternalOutput")
        dbg["kv2"] = nc.dram_tensor("dbg_kv2", [128, S], bf16, kind="ExternalOutput")
        dbg["vext"] = nc.dram_tensor("dbg_vext", [128, S // 128, 65], bf16, kind="ExternalOutput")
        dbg["at"] = nc.dram_tensor("dbg_at", [128, 2, S], bf16, kind="ExternalOutput")
        dbg["rc0"] = nc.dram_tensor("dbg_rc0", [64, 2, 512], f32, kind="ExternalOutput")
        dbg["et0"] = nc.dram_tensor("dbg_et0", [128, 2, 512], bf16, kind="ExternalOutput")

    xt = nc.dram_tensor("xt", [D, S], bf16, kind="ExternalInput")
    wq_t = nc.dram_tensor("wq_t", [D, R * HD], bf16, kind="ExternalInput")
    wkv_t = nc.dram_tensor("wkv_t", [D, 2 * HD], bf16, kind="ExternalInput")
    wo_t = nc.dram_tensor("wo_t", [R * HD, D], bf16, kind="ExternalInput")
    cosb = nc.dram_tensor("cosb", [128, S], bf16, kind="ExternalInput")
    sinb = nc.dram_tensor("sinb", [128, S], bf16, kind="ExternalInput")
    tril = nc.dram_tensor("tril", [128, 128], bf16, kind="ExternalInput")
    ident64 = nc.dram_tensor("ident64", [64, 64], bf16, kind="ExternalInput")
    # rope swap permutations as one-hot stationary matrices:
    # swapm = blockdiag(P64, P64); kswm = [[P64 | I64] , P64]
    swapm = nc.dram_tensor("swapm", [128, 128], bf16, kind="ExternalInput")
    kswm = nc.dram_tensor("kswm", [64, 192], bf16, kind="ExternalInput")
    out_t = nc.dram_tensor("out_t", [D, S], f16, kind="ExternalOutput")

    DT = D // 128  # 16 d tiles
    TT = S // 128  # 16 t blocks
    SB = S // 512  # 4 s superblocks

    with tile.TileContext(nc) as tc:
        with (
            tc.tile_pool(name="persist", bufs=1) as persist,
            tc.tile_pool(name="qstage", bufs=2) as qstage_p,
            tc.tile_pool(name="rtmp", bufs=2) as rtmp_p,
            tc.tile_pool(name="et", bufs=6) as etp,
            tc.tile_pool(name="rcp", bufs=2) as rcp_p,
            tc.tile_pool(name="ostage", bufs=3) as ostage_p,
            tc.tile_pool(name="st_ps", bufs=2, space="PSUM") as st_ps,
            tc.tile_pool(name="ut_ps", bufs=1, space="PSUM") as ut_ps,
            tc.tile_pool(name="qkv_ps", bufs=2, space="PSUM") as qkv_ps,
        ):
            # ---- persistent SBUF tensors ----
            xt_sb = persist.tile([128, DT, S], bf16)
            wq_sb = persist.tile([128, DT, R * HD], bf16)
            wkv_sb = persist.tile([128, DT, 2 * HD], bf16)
            wo_sb = persist.tile([128, 2, D], bf16)
            cos_sb = persist.tile([128, S], bf16)
            sin_sb = persist.tile([128, S], bf16)
            tril_sb = persist.tile([128, 128], bf16)
            ident_sb = persist.tile([128, 64], bf16)
            ones_sb = persist.tile([128, 64], bf16)
            swapm_sb = persist.tile([128, 128], bf16)
            kswm_sb = persist.tile([64, 192], bf16)

            qtr_sb = persist.tile([128, 2, S], bf16)   # roped Q, head-major
            kv2_sb = persist.tile([128, S], bf16)      # 0:64 roped K, 64:128 VT
            ko_sb = persist.tile([128, S], bf16)       # 64:128 roped K (odd heads)
            # V blocks [t, hd]: cols 0:64 = V, col 64 = ones (denominator)
            vext_sb = persist.tile([128, TT, 65], bf16)
            at_sb = persist.tile([128, 2, S], bf16)    # normalized attn out

            # ---- input DMA, ordered for the pipeline lead-in ----
            wkv_r = wkv_t.ap().rearrange("(dt p) j -> p dt j", p=128)
            wq_r = wq_t.ap().rearrange("(dt p) j -> p dt j", p=128)
            xt_r = xt.ap().rearrange("(dt p) s -> p dt s", p=128)
            out_r = out_t.ap().rearrange("(g p) s -> p g s", p=128)

            nc.sync.dma_start(out=wkv_sb, in_=wkv_r)
            nc.sync.dma_start(out=ident_sb[64:128, :], in_=ident64.ap())
            nc.sync.dma_start(out=swapm_sb, in_=swapm.ap())
            nc.sync.dma_start(out=kswm_sb, in_=kswm.ap())
            for dq in range(4):
                nc.sync.dma_start(out=xt_sb[:, 4 * dq:4 * dq + 4, 0:512],
                                  in_=xt_r[:, 4 * dq:4 * dq + 4, 0:512])
            nc.sync.dma_start(out=cos_sb[:, 0:512], in_=cosb.ap()[:, 0:512])
            nc.sync.dma_start(out=sin_sb[:, 0:512], in_=sinb.ap()[:, 0:512])
            nc.sync.dma_start(out=wq_sb, in_=wq_r)
            nc.sync.dma_start(out=tril_sb, in_=tril.ap())
            for sc in range(1, 4):
                c0, c1 = sc * 512, (sc + 1) * 512
                nc.sync.dma_start(out=xt_sb[:, :, c0:c1], in_=xt_r[:, :, c0:c1])
                nc.sync.dma_start(out=cos_sb[:, c0:c1], in_=cosb.ap()[:, c0:c1])
                nc.sync.dma_start(out=sin_sb[:, c0:c1], in_=sinb.ap()[:, c0:c1])
                if sc == 2:
                    for jt in range(2):
                        nc.sync.dma_start(out=wo_sb[:, jt, :], in_=wo_t.ap()[jt * 128:(jt + 1) * 128, :])

            nc.vector.memset(vext_sb, 1.0)  # ones col 64 stays
            nc.vector.memset(ones_sb, 1.0)

            kstage = qstage_p.tile([64, S], bf16, tag="kstage")
            qstage_tiles = [
                qstage_p.tile([128, S], bf16, tag="qstage", name=f"qst{i}")
                for i in range(2)
            ]

            # ---- RoPE: roped = x*C + swap(x)*S, swap done as a PE one-hot
            # matmul (no DMA on the critical path). C=[cos x4],
            # S=[-sin,+sin]x2; swap exchanges rows 0:32 <-> 32:64 per head.
            def rope_q(src, dst, c0, c1):
                sw = qkv_ps.tile([128, 512], f32, tag="mm")
                nc.tensor.matmul(sw, swapm_sb, src[:, c0:c1], start=True, stop=True)
                t1 = rtmp_p.tile([128, 512], bf16, tag="ropetmp")
                nc.vector.tensor_mul(t1, src[:, c0:c1], cos_sb[:, c0:c1])
                t2 = rtmp_p.tile([128, 512], bf16, tag="ropetmp2")
                nc.vector.tensor_mul(t2, sw, sin_sb[:, c0:c1])
                nc.vector.tensor_add(dst, t1, t2)

            # ---- projection chunk: KV proj + K rope + V transpose + Q proj ----
            def proj_chunk(si):
                c0, c1 = si * 512, (si + 1) * 512
                ps = qkv_ps.tile([128, 512], f32, tag="mm")
                for dt in range(DT):
                    nc.tensor.matmul(
                        ps, wkv_sb[:, dt, :], xt_sb[:, dt, c0:c1],
                        start=(dt == 0), stop=(dt == DT - 1),
                    )
                nc.vector.tensor_copy(kstage[:, c0:c1], ps[0:64, :])
                nc.vector.tensor_copy(kv2_sb[64:128, c0:c1], ps[64:128, :])
                # K rope at both PE strips: ps1 = [swap(K) ; K], ps2[64:] = swap(K)
                ps1 = qkv_ps.tile([128, 512], f32, tag="mm")
                nc.tensor.matmul(ps1, kswm_sb[:, 0:128], kstage[:, c0:c1], start=True, stop=True)
                ps2 = qkv_ps.tile([128, 512], f32, tag="mm")
                nc.tensor.matmul(ps2[64:128, :], kswm_sb[:, 128:192], kstage[:, c0:c1], start=True, stop=True)
                t1 = rtmp_p.tile([128, 512], bf16, tag="ropetmp")
                t2 = rtmp_p.tile([128, 512], bf16, tag="ropetmp2")
                nc.vector.tensor_mul(t1[0:64], kstage[:, c0:c1], cos_sb[0:64, c0:c1])
                nc.vector.tensor_mul(t2[0:64], ps1[0:64, :], sin_sb[0:64, c0:c1])
                nc.vector.tensor_add(kv2_sb[0:64, c0:c1], t1[0:64], t2[0:64])
                nc.vector.tensor_mul(t1[64:128], ps1[64:128, :], cos_sb[64:128, c0:c1])
                nc.vector.tensor_mul(t2[64:128], ps2[64:128, :], sin_sb[64:128, c0:c1])
                nc.vector.tensor_add(ko_sb[64:128, c0:c1], t1[64:128], t2[64:128])
                for tt in range(4 * si, 4 * si + 4):
                    vps = st_ps.tile([128, 64], bf16, tag="st")
                    nc.tensor.transpose(vps, kv2_sb[64:128, tt * 128:(tt + 1) * 128], ident_sb[64:128, :])
                    nc.vector.tensor_copy(vext_sb[:, tt, 0:64], vps)
                for jt in range(2):
                    qst = qstage_tiles[jt]
                    ps = qkv_ps.tile([128, 512], f32, tag="mm")
                    for dt in range(DT):
                        nc.tensor.matmul(
                            ps, wq_sb[:, dt, jt * 128:(jt + 1) * 128], xt_sb[:, dt, c0:c1],
                            start=(dt == 0), stop=(dt == DT - 1),
                        )
                    nc.vector.tensor_copy(qst[:, c0:c1], ps)
                    rope_q(qst, qtr_sb[:, jt, c0:c1], c0, c1)

            # ---- attention superblock: heads in pairs (2jt, 2jt+1) ----
            # even head scores at PE contraction rows 0:64, odd at 64:128 ->
            # MM1s overlap in disjoint row strips.
            def att(si):
                nblk = 4 * (si + 1)
                c0, c1 = si * 512, (si + 1) * 512
                for jt in range(2):
                    ut = ut_ps.tile([65, 2, 512], f32, tag="ut")
                    for j in range(nblk):
                        jj = j - 4 * si  # >=0 on the diagonal superblock
                        cc0 = 128 * jj if jj > 0 else 0  # causal col narrowing
                        st2 = st_ps.tile([128, 2, 512], f32, tag="st")
                        nc.tensor.matmul(
                            st2[:, 0, cc0:512],
                            kv2_sb[0:64, j * 128:(j + 1) * 128],
                            qtr_sb[0:64, jt, c0 + cc0:c1],
                            start=True, stop=True,
                        )
                        nc.tensor.matmul(
                            st2[:, 1, cc0:512],
                            ko_sb[64:128, j * 128:(j + 1) * 128],
                            qtr_sb[64:128, jt, c0 + cc0:c1],
                            start=True, stop=True,
                        )
                        et2 = etp.tile([128, 2, 512], bf16, tag="et")
                        nc.scalar.activation(
                            et2[:, :, cc0:512], st2[:, :, cc0:512],
                            mybir.ActivationFunctionType.Exp)
                        if jj >= 0:
                            # triangular band: cols [cc0, cc0+128)
                            nc.vector.tensor_mul(
                                et2[:, 0, cc0:cc0 + 128], et2[:, 0, cc0:cc0 + 128], tril_sb)
                            nc.vector.tensor_mul(
                                et2[:, 1, cc0:cc0 + 128], et2[:, 1, cc0:cc0 + 128], tril_sb)
                        if debug_dump and si == 0 and jt == 0 and j == 0:
                            nc.sync.dma_start(out=dbg["et0"].ap(), in_=et2)
                        # per head: rows 0:64 = V out, row 64 = denominator
                        nc.tensor.matmul(
                            ut[:, 0, cc0:512], vext_sb[:, j, 0:65], et2[:, 0, cc0:512],
                            start=(j == 0), stop=(j == nblk - 1),
                        )
                        nc.tensor.matmul(
                            ut[:, 1, cc0:512], vext_sb[:, j, 0:65], et2[:, 1, cc0:512],
                            start=(j == 0), stop=(j == nblk - 1),
                        )
                    # normalize: at = v_out * (1/den). Evacuate ut whole (so
                    # the PSUM slot frees after one copy); den path runs on
                    # Pool+DVE only so no PE instruction can head-of-line
                    # block on it.
                    utsb = rcp_p.tile([65, 2, 512], f32, tag="utsb")
                    nc.vector.tensor_copy(utsb, ut)
                    den0 = rcp_p.tile([1, 2, 512], f32, tag="den0")
                    nc.gpsimd.dma_start(out=den0, in_=utsb[64:65, :, :])
                    rc = rcp_p.tile([1, 2, 512], f32, tag="rc")
                    nc.vector.reciprocal_approx_fast(rc[:, 0, :], den0[:, 0, :])
                    nc.vector.reciprocal_approx_fast(rc[:, 1, :], den0[:, 1, :])
                    bc0 = rcp_p.tile([64, 512], f32, tag="bc0")
                    bc1 = rcp_p.tile([64, 512], f32, tag="bc1")
                    nc.gpsimd.partition_broadcast(bc0, rc[:, 0, :])
                    nc.gpsimd.partition_broadcast(bc1, rc[:, 1, :])
                    nc.vector.tensor_mul(at_sb[0:64, jt, c0:c1], utsb[0:64, 0, :], bc0)
                    tmp64 = rcp_p.tile([64, 512], bf16, tag="tmp64")
                    nc.vector.tensor_mul(tmp64, utsb[0:64, 1, :], bc1)
                    nc.gpsimd.dma_start(out=at_sb[64:128, jt, c0:c1], in_=tmp64)
                    if debug_dump and si == 0 and jt == 0:
                        nc.sync.dma_start(out=dbg["rc0"].ap(), in_=rc64)

            # ---- partial out-projection for superblock si ----
            def oproj(si):
                c0, c1 = si * 512, (si + 1) * 512
                gsz = 2 if si == SB - 1 else 4  # finer tail granularity
                ost = None
                for dt in range(DT):
                    po = qkv_ps.tile([128, 512], f32, tag="mm")
                    for jt in range(2):
                        nc.tensor.matmul(
                            po, wo_sb[:, jt, dt * 128:(dt + 1) * 128],
                            at_sb[:, jt, c0:c1],
                            start=(jt == 0), stop=(jt == 1),
                        )
                    if dt % gsz == 0:
                        ost = ostage_p.tile([128, gsz, 512], f16, tag="ost")
                    if dt % gsz == gsz - 1:
                        nc.vector.tensor_copy(ost[:, dt % gsz, :], po)
                    else:
                        nc.scalar.activation(ost[:, dt % gsz, :], po, mybir.ActivationFunctionType.Copy)
                    if dt % gsz == gsz - 1:
                        g = dt - gsz + 1
                        nc.sync.dma_start(
                            out=out_r[:, g:g + gsz, c0:c1], in_=ost)

            # ---- emission order == scheduler priority ----
            proj_chunk(0)
            att(0)
            proj_chunk(1)
            att(1)
            oproj(0)
            proj_chunk(2)
            att(2)
            oproj(1)
            proj_chunk(3)
            att(3)
            oproj(2)
            oproj(3)

            if debug_dump:
                nc.sync.dma_start(out=dbg["qtr"].ap(), in_=qtr_sb)
                nc.sync.dma_start(out=dbg["kv2"].ap(), in_=kv2_sb)
                nc.sync.dma_start(out=dbg["vext"].ap(), in_=vext_sb)
                nc.sync.dma_start(out=dbg["at"].ap(), in_=at_sb)

    nc.compile()
    return nc


_SIGMA = np.concatenate([np.arange(0, HD, 2), np.arange(1, HD, 2)])


def _prep_inputs(x, freqs_cis, wq, wk, wv, wo):
    """Host-side shard + layout prep. Returns per-core in_maps."""
    x = np.asarray(x, np.float32).reshape(S, D)
    freqs_cis = np.asarray(freqs_cis, np.float32)
    wq = np.asarray(wq, np.float32)
    wk = np.asarray(wk, np.float32)
    wv = np.asarray(wv, np.float32)
    wo = np.asarray(wo, np.float32)

    xt = np.ascontiguousarray(x.T).astype(BF16)

    cosT = np.ascontiguousarray(freqs_cis[:, :, 0].T)  # [32, S]
    sinT = np.ascontiguousarray(freqs_cis[:, :, 1].T)
    cosb = np.ascontiguousarray(np.tile(cosT, (4, 1))).astype(BF16)
    sinb = np.ascontiguousarray(
        np.concatenate([-sinT, sinT, -sinT, sinT], 0)).astype(BF16)

    tloc = np.arange(128)[:, None]
    cloc = np.arange(128)[None, :]
    tril = (tloc <= cloc).astype(np.float32).astype(BF16)
    ident64 = np.eye(64, dtype=np.float32).astype(BF16)

    # one-hot swap matrices for the PE-matmul rope: P64[k, m] = 1 iff
    # m = (k+32) % 64 (self-inverse pair swap within a 64-row head block)
    P64 = np.zeros((64, 64), np.float32)
    P64[np.arange(64), (np.arange(64) + 32) % 64] = 1.0
    I64 = np.eye(64, dtype=np.float32)
    Z64 = np.zeros((64, 64), np.float32)
    swapm = np.block([[P64, Z64], [Z64, P64]]).astype(BF16)  # [128, 128]
    kswm = np.concatenate([P64, I64, P64], axis=1).astype(BF16)  # [64, 192]

    scale = 1.0 / np.sqrt(HD)
    in_maps = []
    for g in range(NC):
        wqg = wq[g * R * HD:(g + 1) * R * HD].reshape(R, HD, D)[:, _SIGMA, :].reshape(R * HD, D)
        wq_tg = np.ascontiguousarray(wqg.T).astype(BF16)
        wkg = wk[g * HD:(g + 1) * HD][_SIGMA] * scale
        wvg = wv[g * HD:(g + 1) * HD]
        wkv_tg = np.ascontiguousarray(np.concatenate([wkg, wvg], 0).T).astype(BF16)
        wo_tg = np.ascontiguousarray(wo[:, g * R * HD:(g + 1) * R * HD].T).astype(BF16)
        in_maps.append({
            "xt": xt,
            "wq_t": wq_tg,
            "wkv_t": wkv_tg,
            "wo_t": wo_tg,
            "cosb": cosb,
            "sinb": sinb,
            "tril": tril,
            "ident64": ident64,
            "swapm": swapm,
            "kswm": kswm,
        })
    return in_maps


_CACHED = {}


def _get_program():
    if "nc" not in _CACHED:
        _CACHED["nc"] = _build_program()
    return _CACHED["nc"]


def kernel(x, freqs_cis, wq, wk, wv, wo, _trace=False):
    from concourse.bass_utils import run_bass_kernel_spmd

    nc = _get_program()
    in_maps = _prep_inputs(x, freqs_cis, wq, wk, wv, wo)
    res = run_bass_kernel_spmd(nc, in_maps, core_ids=list(range(NC)), trace=_trace)
    acc = np.zeros((D, S), np.float64)
    for c in range(NC):
        acc += res.results[c]["out_t"].astype(np.float64)
    out = np.ascontiguousarray(acc.T, dtype=np.float32).reshape(1, S, D)
    if _trace:
        return out, res
    return out
